# revision 32
# baseline (speedup 1.0000x reference)
"""nn_AttentionC Trainium2 kernel (8 NeuronCores, SPMD).

Sharding: h-axis (64) split into 8 chunks of 8 rows, one per core; each core's
x slab is host-padded to [b2, t10, h10, w72] fp16 tokens (conv zero-padding
baked in). Only cross-core traffic: AllReduce of per-(b,head) [48,48] q/k
gram matrices (110 KB).

Per core (PSUM fp32):
  qkv 1x1 conv on PE -> padded slabs (q/k channels quantized to fp8e4 x16,
  v channels fp16); depthwise 3x3x3:
    q/k: fp8 DoubleRow diag matmuls, two taps per matmul (taps (dt,-1,dw) and
         (dt,+1,dw) differ by 144 B in the slab = 16-aligned pair stride),
         3.6x fewer PE cycles than fp16 diag taps; softmax+normalize washes
         out the fp8 error (measured 5e-4 overall).
    v:   fp16 diag taps (fp8 on the v path fails the 2e-2 gate).
  q~,k~ transposed on PE -> [q;k] grams on PE -> AllReduce -> batched
  norm/softmax on DVE/ACT -> block-diag attn @ v on PE -> proj 1x1 conv on
  PE -> fp32 out.
"""
import numpy as np

DIM = 192
HEADS = 8
HD = DIM // HEADS  # 24
B, T, H, W = 2, 8, 8, 64  # per-core owned h rows = 8
HP, TP = 10, 10
XW = 66  # x staging row width (wpad1 + 64 + wpad1)
WP = 72  # slab row pitch: 64->72 so dh +/-1 tap pairs are 16B apart (fp8)
SLAB = HP * WP  # 720
NTOK = B * T * H * W  # 8192 owned tokens per core
NCORES = 8
C3 = 3 * DIM
NPADTOK = B * TP * HP * XW  # 13200 (x staging tokens, 66-wide rows)
ASCALE = 16.0  # fp8 slab scale
WSCALE = 64.0  # fp8 diag scale
DW_DEQ = 1.0 / (ASCALE * WSCALE)

_CACHE = {}

MTILES = [(0, 128), (128, 128), (256, 128), (384, 128), (512, 64)]
KTILES = [(0, 128), (128, 64)]
TAPS = [(dt, dh, dw) for dt in (-1, 0, 1) for dh in (-1, 0, 1)
        for dw in (-1, 0, 1)]
# fp8 DoubleRow pair plan for q/k, 21 lhsT tiles per mtile:
#  j 0..8   within-plane pairs (dt,-1,dw)+(dt,+1,dw), j = (dt+1)*3 + dw+1
#  j 9..11  center singles (0,0,dw) with zero slot B
#  j 12..14 cross-plane (-1,0,dw)+(+1,0,dw)    [interior chunks]
#  j 15..17 cross-plane (0,0,dw)+(+1,0,dw)     [t_o == 0]
#  j 18..20 cross-plane (-1,0,dw)+(0,0,dw)     [t_o == T-1]
NQK_TILES = 21  # per mtile

# ---- v depthwise engine plan ----
# m3 (v channels 384..511): 27 fp16 diag taps, split PE / DVE(ts+tt) /
# Act(mult)+DVE(add) / DVE(mult)+Pool(add).
# m4 (v channels 512..575, 64ch): slab ring duplicated into partitions
# 64..127 shifted by -2 rows, so (dt,-1,dw)+(dt,+1,dw) pairs become ONE
# 128-contraction matmul on PE; dh=0 taps stay singles.
# Tap lists are ordered so edge chunks (dropped dt planes) thin every
# engine roughly equally.
M3_TAPS = [(dt, dh, dw) for dt in (0, -1, 1) for dh in (-1, 0, 1)
           for dw in (-1, 0, 1)]
M3_PE, M3_ACT, M3_POOL = 2, 6, 4  # counts; rest -> DVE
M4_PAIRS = [(dt, dw) for dt in (0, -1, 1) for dw in (-1, 0, 1)]  # PE
M4_SINGLES = [(dt, 0, dw) for dt in (0, -1, 1) for dw in (-1, 0, 1)]
M4_PE, M4_ACT, M4_POOL = 0, 2, 2  # singles counts; rest -> DVE
# qkv-conv psum eviction engines per (mtile, half) -> 'a'(Act) 'd'(DVE).
# GPSIMD/Pool cannot touch PSUM, so evictions split Act/DVE only;
# m4 entries are (base, dup) pairs.
EV_QK = ['a', 'd', 'a', 'd', 'a', 'd']   # (mi, half) for mi 0..2
EV_M3 = ['d', 'a']
EV_M4 = [('d', 'a'), ('a', 'd')]         # (base, dup) per half


def _build():
    import concourse.bacc as bacc
    import concourse.mybir as mybir
    import concourse.tile as tile
    from concourse import masks
    from concourse.ap import AP
    import bass_rust

    F32 = mybir.dt.float32
    F16 = mybir.dt.float16
    F8 = mybir.dt.float8e4
    AL = mybir.AluOpType
    AF = mybir.ActivationFunctionType
    AX = mybir.AxisListType
    DR = mybir.MatmulPerfMode.DoubleRow

    nc = bacc.Bacc("TRN2", target_bir_lowering=False, debug=False,
                   num_devices=NCORES)

    # fp16 x for the v half of the 1x1 conv (fp8 v fails the 2e-2 gate)
    x16 = nc.dram_tensor("x16", [DIM, NPADTOK], F16, kind="ExternalInput").ap()
    # fp8 copy of x, 192 channels as 2 k-tiles of 96 in the same partitions
    # (DoubleRow contraction for the q/k half of the 1x1 conv)
    x8 = nc.dram_tensor("x8", [96, 2 * NPADTOK], F8, kind="ExternalInput").ap()
    # q/k 1x1 conv weights fp8 [96, 2, 128] per qk mtile
    wq8d = nc.dram_tensor("wq8d", [96, 3 * 256], F8, kind="ExternalInput").ap()
    # fp16 v-conv weights, [kc, 256] per ktile side by side:
    # cols 0..127 m3, cols 128..255 m4 (64ch duplicated)
    wv16d = nc.dram_tensor("wv16d", [128, 512], F16,
                           kind="ExternalInput").ap()
    qkvb = nc.dram_tensor("qkvb", [128, 5], F32, kind="ExternalInput").ap()
    # per-channel f32 v-diag values for the Pool-engine taps [128, 2*27]
    vdws = nc.dram_tensor("vdws", [128, 54], F32, kind="ExternalInput").ap()
    # fp8 DoubleRow diag-pair tiles for q/k: 3 mtiles x 3 planes x 6 tiles,
    # each [128, 2, 128] fp8 (values 64*d on the diagonal)
    qkdiag = nc.dram_tensor("qkdiag", [128, 3 * NQK_TILES * 256], F8,
                            kind="ExternalInput").ap()
    # fp16 exact diag tiles for v (mtile 3: 128ch, mtile 4: 64ch)
    vdiag3 = nc.dram_tensor("vdiag3", [128, 27 * 128], F16,
                            kind="ExternalInput").ap()
    # only the 9 dh=0 taps are ever applied as m4 singles
    vdiag4 = nc.dram_tensor("vdiag4", [64, 9 * 64], F16,
                            kind="ExternalInput").ap()
    # stacked dh=-1/dh=+1 diag pairs for the dup'd m4 ring [128, 9*64]
    vdiag4p = nc.dram_tensor("vdiag4p", [128, 9 * 64], F16,
                             kind="ExternalInput").ap()
    dwb = nc.dram_tensor("dwb", [128, 5], F32, kind="ExternalInput").ap()
    # proj weights restaged per head: projr[c, 192h+m] = proj_w[m, 24h+c]
    projr = nc.dram_tensor("projr", [HD, HEADS * DIM], F16,
                           kind="ExternalInput").ap()
    projb = nc.dram_tensor("projb", [128, 2], F32, kind="ExternalInput").ap()
    temp = nc.dram_tensor("temp", [8, 2], F32, kind="ExternalInput").ap()
    out = nc.dram_tensor("out", [DIM, NTOK], F32, kind="ExternalOutput").ap()

    gram_in = nc.dram_tensor("gram_in", [16, 48, 48], F32).ap()
    gram_out = nc.dram_tensor("gram_out", [16, 48, 48], F32,
                              addr_space="Shared").ap()
    attn_dram = nc.dram_tensor("attn_dram", [16, HD, HD], F16).ap()

    with tile.TileContext(nc) as tc:
        with (
            tc.tile_pool(name="wp", bufs=1) as wp,
            tc.tile_pool(name="xp", bufs=3) as xp,
            tc.tile_pool(name="qslab", bufs=5) as slp,
            tc.tile_pool(name="qk", bufs=1) as qkpool,
            tc.tile_pool(name="ev", bufs=3) as ev,
            tc.tile_pool(name="small", bufs=1) as sp,
            tc.tile_pool(name="ps", bufs=4, space="PSUM") as psp,
            tc.tile_pool(name="psav", bufs=2, space="PSUM") as psav,
            tc.tile_pool(name="psg", bufs=1, space="PSUM") as psg,
        ):
            # ---------------- weights ----------------
            wq8 = wp.tile([96, 3 * 256], F8, tag="wq8")
            nc.sync.dma_start(out=wq8[:], in_=wq8d)
            wv16 = wp.tile([128, 512], F16, tag="wv16")
            nc.gpsimd.dma_start(out=wv16[:], in_=wv16d)
            vdws_s = wp.tile([128, 54], F32, tag="vdws")
            nc.scalar.dma_start(out=vdws_s[:], in_=vdws)
            qkvb_s = wp.tile([128, 5], F32, tag="qkvb")
            nc.sync.dma_start(out=qkvb_s[:], in_=qkvb)
            qkdiag_s = wp.tile([128, 3 * NQK_TILES * 256], F8,
                               tag="qkdiag")
            nc.scalar.dma_start(out=qkdiag_s[:], in_=qkdiag)
            vd3 = wp.tile([128, 27 * 128], F16, tag="vd3")
            nc.scalar.dma_start(out=vd3[:], in_=vdiag3)
            vd4 = wp.tile([64, 9 * 64], F16, tag="vd4")
            nc.scalar.dma_start(out=vd4[:], in_=vdiag4)
            vd4p_s = wp.tile([128, 9 * 64], F16, tag="vd4p")
            nc.gpsimd.dma_start(out=vd4p_s[:], in_=vdiag4p)
            dwb_s = wp.tile([128, 5], F32, tag="dwb")
            nc.scalar.dma_start(out=dwb_s[:], in_=dwb)
            projr_s = wp.tile([HD, HEADS * DIM], F16, tag="projr")
            nc.scalar.dma_start(out=projr_s[:], in_=projr)
            projb_s = wp.tile([128, 2], F32, tag="projb")
            nc.sync.dma_start(out=projb_s[:], in_=projb)
            temp_s = wp.tile([8, 2], F32, tag="temp")
            nc.sync.dma_start(out=temp_s[:], in_=temp)

            ident16 = wp.tile([128, 128], F16, tag="ident16")
            masks.make_identity(nc, ident16[:])

            # dw outputs: v (192 ch) in 2 materialized tiles; q~/k~ go
            # through per-chunk ring tiles + XBAR DMA transpose into
            # qkT_all [tok128, chunk64, ch384]
            v_t = [qkpool.tile([vc, NTOK], F16, tag=f"v{i}", name=f"v{i}")
                   for i, vc in enumerate([128, 64])]
            qkT_all = qkpool.tile([128, 64, 384], F16, tag="qkT_all",
                                  name="qkT_all")
            # q/k slab rings: all 5 slots in one tensor so DoubleRow pair
            # strides may cross dt planes (slot pitch 720, 16-aligned)
            slring = [qkpool.tile([128, 5, SLAB], F8, tag=f"slr{mi}",
                                  name=f"slr{mi}") for mi in range(3)]
            # v m4 ring: partitions 0..63 = slab, 64..127 = slab shifted
            # -2 rows (so dh=-1/dh=+1 tap pairs become one PE matmul)
            ring4 = qkpool.tile([128, 5, SLAB], F16, tag="ring4",
                                name="ring4")

            def evict(engine, out_ap, in_ap, bias, scale=1.0):
                if engine == 'a':
                    nc.scalar.activation(out_ap, in_ap, AF.Identity,
                                         bias=bias, scale=scale)
                elif engine == 'd':
                    nc.vector.tensor_scalar(out_ap, in_ap, scale, bias,
                                            AL.mult, AL.add)
                else:
                    nc.gpsimd.tensor_scalar(out_ap, in_ap, scale, bias,
                                            AL.mult, AL.add)

            # ---------------- qkv conv + depthwise ----------------
            # x staging rows are 66 wide; slab rows are 72 wide (alignment
            # pad).  conv output written as [5 rows x 66] halves; q/k slab
            # cols 66..71 of each row are memset once per slab (fp8 DR
            # zero-slot singles read +16 past the data cols).
            def qkv_slab(b, t_, slabs):
                    xoff = (b * TP + t_) * HP * XW
                    # slot pitch 672 (16-aligned) holding 660 data cols
                    xx8 = xp.tile([96, 2, 672], F8, tag="x8")
                    nc.sync.dma_start(
                        out=xx8[:, :, 0:HP * XW],
                        in_=x8.rearrange("p (i n) -> p i n",
                                         i=2)[:, :, xoff:xoff + HP * XW])
                    xt = []
                    for ko, kc in KTILES:
                        xx = xp.tile([kc, HP * XW], F16, tag=f"x{ko}")
                        nc.sync.dma_start(
                            out=xx[:],
                            in_=x16[ko:ko + kc, xoff:xoff + HP * XW])
                        xt.append(xx)
                    mts = []
                    # q/k mtiles: fp8 DoubleRow conv
                    for mi in range(3):
                        sl = slring[mi][:, t_ % 5, :]
                        slr = sl.rearrange("p (h w) -> p h w", h=HP)
                        nc.gpsimd.memset(slr[:, :, XW:WP], 0.0)
                        for half in range(2):
                            ps = psp.tile([128, 512], F32, tag="mm")
                            pd = xx8.ap[0]
                            rhs = AP(tensor=xx8.tensor,
                                     offset=xx8.offset + 330 * half,
                                     ap=bass_rust.VecI64Pair(
                                         [[pd[0], pd[1]],
                                          [672, 2], [1, 330]]))
                            nc.tensor.matmul(
                                ps[:128, :330],
                                wq8[:, 256 * mi:256 * (mi + 1)]
                                .rearrange("p (i m) -> p i m", i=2),
                                rhs, start=True, stop=True, perf_mode=DR)
                            evict(EV_QK[2 * mi + half],
                                  slr[:, 5 * half:5 * (half + 1), 0:XW],
                                  ps[:128, :330].rearrange(
                                      "p (h w) -> p h w", h=5),
                                  qkvb_s[:128, mi:mi + 1], ASCALE)
                        mts.append(sl)
                    # v mtile 3: fp16 conv, 128 ch
                    sl3 = slp.tile([128, SLAB], F16, tag="sl3")
                    sl3r = sl3.rearrange("p (h w) -> p h w", h=HP)
                    for half in range(2):
                        ps = psp.tile([128, 512], F32, tag="mm")
                        for ki, (ko, kc) in enumerate(KTILES):
                            nc.tensor.matmul(
                                ps[:128, :330],
                                wv16[0:kc, 256 * ki:256 * ki + 128],
                                xt[ki][:, 330 * half:330 * (half + 1)],
                                start=(ki == 0), stop=(ki == 1))
                        evict(EV_M3[half],
                              sl3r[:, 5 * half:5 * (half + 1), 0:XW],
                              ps[:128, :330].rearrange("p (h w) -> p h w",
                                                       h=5),
                              qkvb_s[:128, 3:4])
                    mts.append(sl3)
                    # v mtile 4: fp16 conv with duplicated weights; psum
                    # partitions 64..127 hold the same 64 channels, evicted
                    # at -2 rows into the dup half of ring4
                    r4 = ring4[:, t_ % 5, :]
                    r4v = r4.rearrange("p (h w) -> p h w", h=HP)
                    for half in range(2):
                        ps = psp.tile([128, 512], F32, tag="mm")
                        for ki, (ko, kc) in enumerate(KTILES):
                            nc.tensor.matmul(
                                ps[:128, :330],
                                wv16[0:kc, 256 * ki + 128:256 * ki + 256],
                                xt[ki][:, 330 * half:330 * (half + 1)],
                                start=(ki == 0), stop=(ki == 1))
                        psv = ps[:, :330].rearrange("p (h w) -> p h w", h=5)
                        evict(EV_M4[half][0],
                              r4v[0:64, 5 * half:5 * (half + 1), 0:XW],
                              psv[0:64], qkvb_s[0:64, 4:5])
                        if half == 0:
                            evict(EV_M4[half][1], r4v[64:128, 0:3, 0:XW],
                                  psv[64:128, 2:5], qkvb_s[64:128, 4:5])
                        else:
                            evict(EV_M4[half][1], r4v[64:128, 3:8, 0:XW],
                                  psv[64:128], qkvb_s[64:128, 4:5])
                    mts.append(r4)
                    slabs[t_] = mts

            def pair_rhs(src, offA, delta):
                """[128, 2, 8, 64] view of the fp8 slab: slot i at
                offA+i*delta, then 8 rows of 64 at pitch WP."""
                pd = src.ap[0]
                return AP(tensor=src.tensor, offset=src.offset + offA,
                          ap=bass_rust.VecI64Pair(
                              [[pd[0], pd[1]], [delta, 2], [WP, 8], [1, 64]]))

            def win(src, row, dwv, mc):
                """[mc, 8, 64] window of a slab at given start row/w shift."""
                return src[:mc].rearrange(
                    "p (h w) -> p h w", h=HP)[:, row:row + 8,
                                              1 + dwv:65 + dwv]

            def gram_chunk(b, chunk):
                for c64 in range(4 * chunk, 4 * (chunk + 1)):
                    for h in range(HEADS):
                        z = qkT_all[:, c64, 48 * h:48 * (h + 1)]
                        nc.tensor.matmul(
                            gps[b][:, 48 * h:48 * (h + 1)], z, z,
                            start=(c64 == 32 * b and h == 0),
                            stop=(c64 == 32 * b + 31 and h == HEADS - 1))

            def dw_chunk(b, t_o, slabs):
                    chunk = b * T + t_o
                    co = 512 * chunk
                    planes = [dt for dt in (-1, 0, 1)
                              if not ((t_o == 0 and dt == -1) or
                                      (t_o == T - 1 and dt == 1))]
                    # q/k mtiles: fp8 DoubleRow with within-plane and
                    # cross-plane pair strides on the slab ring
                    slot = {dt: (t_o + 1 + dt) % 5 for dt in (-1, 0, 1)}
                    mms = []  # (tile j, slotA, offA, delta)
                    for dt in planes:
                        for dw in (-1, 0, 1):
                            mms.append(((dt + 1) * 3 + dw + 1, slot[dt],
                                        1 + dw, 2 * WP))
                    if len(planes) == 3:
                        for dw in (-1, 0, 1):
                            mms.append((12 + dw + 1, slot[-1], WP + 1 + dw,
                                        (slot[1] - slot[-1]) * SLAB))
                            mms.append((9 + dw + 1, slot[0], WP + 1 + dw,
                                        16))
                    elif t_o == 0:
                        for dw in (-1, 0, 1):
                            mms.append((15 + dw + 1, slot[0], WP + 1 + dw,
                                        (slot[1] - slot[0]) * SLAB))
                    else:
                        for dw in (-1, 0, 1):
                            mms.append((18 + dw + 1, slot[-1], WP + 1 + dw,
                                        (slot[0] - slot[-1]) * SLAB))
                    for mi in range(3):
                        ring = slring[mi]
                        pd = ring.ap[0]
                        ps = psp.tile([128, 512], F32, tag="mm")
                        for j, (tj, sA, offA, delta) in enumerate(mms):
                            ti = mi * NQK_TILES + tj
                            rhs = AP(tensor=ring.tensor,
                                     offset=ring.offset + sA * SLAB + offA,
                                     ap=bass_rust.VecI64Pair(
                                         [[pd[0], pd[1]], [delta, 2],
                                          [WP, 8], [1, 64]]))
                            nc.tensor.matmul(
                                ps[:128, :512],
                                qkdiag_s[:, 256 * ti:256 * (ti + 1)]
                                .rearrange("p (i m) -> p i m", i=2),
                                rhs, start=(j == 0), stop=(j == len(mms) - 1),
                                perf_mode=DR)
                        qkc = ev.tile([128, 512], F16, tag=f"qkc{mi}",
                                      name=f"qkc{mi}")
                        nc.scalar.activation(
                            qkc[:], ps[:128, :512],
                            AF.Identity, bias=dwb_s[:128, mi:mi + 1],
                            scale=DW_DEQ)
                        qdma = (nc.sync, nc.scalar, nc.sync)[mi]
                        qdma.dma_start_transpose(
                            out=qkT_all[:, 4 * chunk:4 * (chunk + 1),
                                        128 * mi:128 * (mi + 1)],
                            in_=qkc[:])
                    # grams for the PREVIOUS chunk (its transpose DMAs have
                    # had a full chunk of time to land; PE is in-order so a
                    # not-yet-ready gram matmul would stall the dw stream)
                    if t_o > 0:
                        gram_chunk(b, chunk - 1)
                    # ---- v depthwise, engine-split per module plan ----
                    def vsplit(taps, n_pe, n_act, n_pool):
                        pe = taps[:n_pe]
                        act = taps[n_pe:n_pe + n_act]
                        pool = taps[n_pe + n_act:n_pe + n_act + n_pool]
                        dve = taps[n_pe + n_act + n_pool:]
                        return pe, act, pool, dve

                    def vtap_nonpe(vi, mi, mc, src_of, act_taps, pool_taps,
                                   dve_taps):
                        """DVE acc chain + Act-mult and Pool-add offloads.
                        Returns acc tile (or None if no non-PE taps)."""
                        seq = ([('d', t) for t in dve_taps] +
                               [('a', t) for t in act_taps] +
                               [('p', t) for t in pool_taps])
                        if not seq:
                            return None
                        acc = ev.tile([mc, 512], F16, tag=f"vacc{vi}",
                                      name=f"acc{vi}")
                        accv = acc[:].rearrange("p (h w) -> p h w", h=8)
                        for oj, (eng, (dt, dh, dwv)) in enumerate(seq):
                            ti = TAPS.index((dt, dh, dwv))
                            dcol = vdws_s[:mc,
                                          27 * vi + ti:27 * vi + ti + 1]
                            w_in = win(src_of(dt), 1 + dh, dwv, mc)
                            if oj == 0:
                                nc.vector.tensor_scalar(
                                    accv, w_in, dcol, None, AL.mult)
                                continue
                            tmp = ev.tile([mc, 512], F16, tag=f"vtmp{vi}",
                                          name=f"tmp{vi}")
                            tmpv = tmp[:].rearrange("p (h w) -> p h w", h=8)
                            if eng == 'a':
                                nc.scalar.activation(
                                    tmpv, w_in, AF.Identity, scale=dcol)
                            else:
                                nc.vector.tensor_scalar(
                                    tmpv, w_in, dcol, None, AL.mult)
                            addq = nc.gpsimd if eng == 'p' else nc.vector
                            addq.tensor_tensor(acc[:], acc[:], tmp[:],
                                               AL.add)
                        return acc

                    def vmerge(vi, mi, mc, acc, ps, n_pe_taps):
                        dst = v_t[vi][:, co:co + 512]
                        bias = dwb_s[:mc, mi:mi + 1]
                        if acc is None:
                            nc.scalar.activation(dst, ps[:mc, :512],
                                                 AF.Identity, bias=bias)
                        elif n_pe_taps == 0:
                            nc.vector.tensor_scalar(dst, acc[:], 1.0, bias,
                                                    AL.mult, AL.add)
                        else:
                            tmpm = ev.tile([mc, 512], F16, tag=f"vmrg{vi}",
                                           name=f"mrg{vi}")
                            nc.scalar.activation(tmpm[:], ps[:mc, :512],
                                                 AF.Identity, bias=bias)
                            nc.vector.tensor_tensor(dst, acc[:], tmpm[:],
                                                    AL.add)

                    tvalid = lambda dt: dt in planes
                    # m3: 27 diag taps
                    taps3 = [t for t in M3_TAPS if tvalid(t[0])]
                    pe3, act3, pool3, dve3 = vsplit(taps3, M3_PE, M3_ACT,
                                                    M3_POOL)
                    src3 = lambda dt: slabs[t_o + 1 + dt][3]
                    acc3 = vtap_nonpe(0, 3, 128, src3, act3, pool3, dve3)
                    ps3 = None
                    if pe3:
                        ps3 = psp.tile([128, 512], F32, tag="mm")
                        for j, (dt, dh, dwv) in enumerate(pe3):
                            ti = TAPS.index((dt, dh, dwv))
                            nc.tensor.matmul(
                                ps3[:128, :512],
                                vd3[:, 128 * ti:128 * (ti + 1)],
                                win(src3(dt), 1 + dh, dwv, 128),
                                start=(j == 0), stop=(j == len(pe3) - 1))
                    vmerge(0, 3, 128, acc3, ps3, len(pe3))
                    # m4: PE pairs on the dup'd ring + split singles
                    pairs4 = [(dt, dw) for dt, dw in M4_PAIRS if tvalid(dt)]
                    sing4 = [t for t in M4_SINGLES if tvalid(t[0])]
                    pe4, act4, pool4, dve4 = vsplit(sing4, M4_PE, M4_ACT,
                                                    M4_POOL)
                    src4 = lambda dt: slabs[t_o + 1 + dt][4][0:64, :]
                    acc4 = vtap_nonpe(1, 4, 64, src4, act4, pool4, dve4)
                    ps4 = psp.tile([128, 512], F32, tag="mm")
                    n4 = len(pairs4) + len(pe4)
                    pd4 = ring4.ap[0]
                    for j, (dt, dwv) in enumerate(pairs4):
                        pidx = M4_PAIRS.index((dt, dwv))
                        src = slabs[t_o + 1 + dt][4]
                        rhs = AP(tensor=src.tensor,
                                 offset=src.offset + 1 + dwv,
                                 ap=bass_rust.VecI64Pair(
                                     [[pd4[0], 128], [WP, 8], [1, 64]]))
                        nc.tensor.matmul(
                            ps4[:64, :512],
                            vd4p_s[:, 64 * pidx:64 * (pidx + 1)], rhs,
                            start=(j == 0), stop=(j == n4 - 1))
                    for j, (dt, dh, dwv) in enumerate(pe4):
                        si = M4_SINGLES.index((dt, dh, dwv))
                        nc.tensor.matmul(
                            ps4[:64, :512], vd4[:, 64 * si:64 * (si + 1)],
                            win(src4(dt), 1 + dh, dwv, 64),
                            start=(len(pairs4) + j == 0),
                            stop=(len(pairs4) + j == n4 - 1))
                    vmerge(1, 4, 64, acc4, ps4, n4)

            # per-batch norms + softmax + attn@v + proj (emitted after each
            # batch's AllReduce so batch 0's tail overlaps batch 1's dw)
            def attn_batch(b):
                # diagonals of the q/k gram blocks, loaded strided from dram
                qq_d = sp.tile([8, 24], F32, tag="qqd", name="qq_d")
                kk_d = sp.tile([8, 24], F32, tag="kkd", name="kk_d")
                qk_f = sp.tile([8, 576], F32, tag="qkf", name="qk_f")
                g8 = gram_out[8 * b:8 * (b + 1)]
                nc.sync.dma_start(
                    out=qq_d[:],
                    in_=AP(tensor=g8.tensor, offset=g8.offset,
                           ap=bass_rust.VecI64Pair([[2304, 8], [49, 24]])))
                nc.scalar.dma_start(
                    out=kk_d[:],
                    in_=AP(tensor=g8.tensor, offset=g8.offset + 24 * 48 + 24,
                           ap=bass_rust.VecI64Pair([[2304, 8], [49, 24]])))
                nc.sync.dma_start(
                    out=qk_f[:].rearrange("p (c d) -> p c d", c=24),
                    in_=g8[:, 0:24, 24:48])

                def diag_rsqrt(src, tag):
                    nrm = sp.tile([8, 24], F32, tag=tag + "c", name="nrm")
                    nc.scalar.sqrt(nrm[:], src[:])
                    nc.vector.tensor_scalar_max(nrm[:], nrm[:], 1e-12)
                    r = sp.tile([8, 24], F32, tag=tag + "d", name="r")
                    nc.vector.reciprocal(r[:], nrm[:])
                    return r

                rq = diag_rsqrt(qq_d, "rq")
                rk = diag_rsqrt(kk_d, "rk")

                # logits = qk_gram * rq_c * rk_d * temp; |logit| <= temp so
                # exp() is overflow-safe without the max subtraction
                a1 = sp.tile([8, 576], F32, tag="a1", name="a1")
                nc.vector.tensor_mul(
                    a1[:].rearrange("p (c d) -> p c d", c=24),
                    qk_f[:].rearrange("p (c d) -> p c d", c=24),
                    rq[:].rearrange("p (c one) -> p c one",
                                    one=1).broadcast_to((8, 24, 24)))
                nc.vector.tensor_mul(
                    a1[:].rearrange("p (c d) -> p c d", c=24),
                    a1[:].rearrange("p (c d) -> p c d", c=24),
                    rk[:].rearrange("p (one d) -> p one d",
                                    one=1).broadcast_to((8, 24, 24)))
                nc.vector.tensor_scalar_mul(a1[:], a1[:], temp_s[:, b:b + 1])
                ex = sp.tile([8, 576], F32, tag="ex", name="ex")
                nc.scalar.activation(ex[:], a1[:], AF.Exp)
                sm = sp.tile([8, 24], F32, tag="sm", name="sm")
                nc.vector.tensor_reduce(
                    sm[:], ex[:].rearrange("p (c d) -> p c d", c=24),
                    axis=AX.X, op=AL.add)
                rs = sp.tile([8, 24], F32, tag="rs", name="rs")
                nc.vector.reciprocal(rs[:], sm[:])
                at16 = sp.tile([8, 576], F16, tag="at16", name="at16")
                nc.vector.tensor_mul(
                    at16[:].rearrange("p (c d) -> p c d", c=24),
                    ex[:].rearrange("p (c d) -> p c d", c=24),
                    rs[:].rearrange("p (c one) -> p c one",
                                    one=1).broadcast_to((8, 24, 24)))

                # W2 = proj @ blockdiag(attn): per head a [24ch, 24] matmul
                # with the host-restaged projr lhsT, then transpose to get
                # the k-major lhsT for the fused (attn@v+proj) stage
                a_rhs = sp.tile([24, 8, 24], F16, tag=f"ar{b}", name="a_rhs")
                for h in range(HEADS):
                    q = nc.sync if h % 2 == 0 else nc.scalar
                    q.dma_start(
                        out=a_rhs[:, h, :],
                        in_=at16[h:h + 1, :].rearrange(
                            "p (c d) -> p c d", c=24))
                w2m = []
                for mi, (mo, mc) in enumerate(KTILES):
                    pw = psav.tile([128, 512], F32, tag="av")
                    for h in range(HEADS):
                        nc.tensor.matmul(
                            pw[:mc, 24 * h:24 * (h + 1)],
                            projr_s[:, 192 * h + mo:192 * h + mo + mc],
                            a_rhs[:, h, :],
                            start=(h == 0), stop=(h == HEADS - 1))
                    wm = sp.tile([mc, DIM], F16, tag=f"w2m{b}_{mi}",
                                 name="wm")
                    nc.scalar.activation(wm[:], pw[:mc, :DIM], AF.Identity)
                    w2m.append(wm)
                w2T = []
                for ki, (ko, kc) in enumerate(KTILES):
                    wt = sp.tile([kc, DIM], F16, tag=f"w2T{b}_{ki}",
                                 name="wt")
                    for mi, (mo, mc) in enumerate(KTILES):
                        pt = psav.tile([128, 512], F32, tag="av")
                        ptv = pt.bitcast(F16)[:kc, :mc]
                        nc.tensor.transpose(
                            ptv, w2m[mi][:, ko:ko + kc], ident16[:mc, :mc])
                        nc.scalar.activation(wt[:, mo:mo + mc], ptv,
                                             AF.Identity)
                    w2T.append(wt)
                return w2T

            def av_proj(w2T, chunks):
                for chunk in chunks:
                    co = 512 * chunk
                    for mi, (mo, mc) in enumerate(KTILES):
                        ps = psav.tile([128, 512], F32, tag="av")
                        for ki in range(2):
                            nc.tensor.matmul(
                                ps[:mc, :], w2T[ki][:, mo:mo + mc],
                                v_t[ki][:, co:co + 512],
                                start=(ki == 0), stop=(ki == 1))
                        of = ev.tile([128, 512], F32, tag="of")
                        if mi == 0:
                            nc.vector.tensor_scalar(
                                of[:mc, :], ps[:mc, :],
                                projb_s[:mc, mi:mi + 1], None, AL.add)
                        else:
                            nc.scalar.activation(
                                of[:mc, :], ps[:mc, :], AF.Identity,
                                bias=projb_s[:mc, mi:mi + 1])
                        nc.sync.dma_start(out=out[mo:mo + mc, co:co + 512],
                                          in_=of[:mc, :])

            gps = [psg.tile([48, 384], F32, tag=f"gram{i}", name=f"gram{i}") for i in range(2)]
            dm = sp.tile([8, 576], F32, tag="dm")
            nc.gpsimd.memset(dm[:], 0.0)
            nc.gpsimd.affine_select(
                out=dm[:], in_=dm[:], compare_op=AL.not_equal, fill=1.0,
                base=0, pattern=[[1, 24], [-1, 24]], channel_multiplier=0)
            # emission order == per-engine execution order (engines run
            # their streams in order), so nothing that waits on a collective
            # may be emitted ahead of ready work:
            #  b0 dw -> collective(0) -> b1 dw chunk 0 -> b0 softmax/W2 +
            #  av(0, 0..5) -> b1 dw rest -> collective(1) -> av(0, 5..8)
            #  (fills the collective-1 latency) -> b1 softmax/W2 -> av(1)
            w2T0 = None
            for b in range(B):
                slabs = {}
                for t_ in (1, 2, 3):
                    qkv_slab(b, t_, slabs)
                for t_o in range(T):
                    if t_o + 4 <= T:
                        qkv_slab(b, t_o + 4, slabs)
                    dw_chunk(b, t_o, slabs)
                    if b == 1 and t_o == 1:
                        w2T0 = attn_batch(0)
                        av_proj(w2T0, range(0, 5))
                gram_chunk(b, b * T + T - 1)
                gs = ev.tile([48, 384], F32, tag="gs")
                nc.vector.tensor_copy(gs[:], gps[b][:])
                nc.sync.dma_start(
                    out=gram_in[8 * b:8 * (b + 1)].rearrange(
                        "g c d -> c g d"),
                    in_=gs[:].rearrange("c (g d) -> c g d", g=8))
                nc.gpsimd.collective_compute(
                    "AllReduce", AL.add,
                    replica_groups=[list(range(NCORES))],
                    ins=[gram_in[8 * b:8 * (b + 1)]],
                    outs=[gram_out[8 * b:8 * (b + 1)]])
            av_proj(w2T0, range(5, T))
            w2T1 = attn_batch(1)
            av_proj(w2T1, range(T, 2 * T))

    nc.compile()
    return nc


def _prep_inputs(x, qkv_w, qkv_b, dw_w, dw_b, temperature, proj_w, proj_b):
    """Host-side prep: per-core padded fp16 slabs + shared weights."""
    x = np.asarray(x, np.float32)
    b_, c_, t_, h_, w_ = x.shape  # 2, 192, 8, 64, 64
    qkv_w2 = np.asarray(qkv_w, np.float32).reshape(C3, DIM)
    dw_w2 = np.asarray(dw_w, np.float32).reshape(C3, 27)
    proj_w2 = np.asarray(proj_w, np.float32).reshape(DIM, DIM)
    # permute qkv channels: [q_h0, k_h0, q_h1, k_h1, ..., v] so each head's
    # (q,k) columns are adjacent after transpose (contiguous gram operands)
    perm = []
    for h in range(HEADS):
        perm.extend(range(HD * h, HD * (h + 1)))          # q_h
        perm.extend(range(DIM + HD * h, DIM + HD * (h + 1)))  # k_h
    perm.extend(range(2 * DIM, 3 * DIM))                  # v unchanged
    perm = np.array(perm)
    qkv_w2 = qkv_w2[perm]
    dw_w2 = dw_w2[perm]
    qkv_b = np.asarray(qkv_b, np.float32)[perm]
    dw_b = np.asarray(dw_b, np.float32)[perm]

    import ml_dtypes
    FP8 = ml_dtypes.float8_e4m3

    qkvb_h = np.zeros((128, 5), np.float32)
    dwb_h = np.zeros((128, 5), np.float32)
    for mi, (mo, mc) in enumerate(MTILES):
        s = ASCALE if mi < 3 else 1.0  # qk slab evicted as fp8(ASCALE*psum)
        qkvb_h[:mc, mi] = np.asarray(qkv_b, np.float32)[mo:mo + mc] * s
        dwb_h[:mc, mi] = np.asarray(dw_b, np.float32)[mo:mo + mc]
    qkvb_h[64:128, 4] = qkvb_h[0:64, 4]  # dup half of the m4 ring

    # fp8 DoubleRow diag-pair tiles for q/k (values WSCALE*d, fp8-rounded)
    tap_i = {tap: i for i, tap in enumerate(TAPS)}
    qkd = np.zeros((128, 3 * NQK_TILES * 256), FP8)
    d8 = (WSCALE * dw_w2).astype(FP8)  # [576, 27]
    rng = np.arange(128)

    def put(mi, j, slot, tap):
        base = 256 * (mi * NQK_TILES + j) + 128 * slot
        qkd[rng, base + rng] = d8[128 * mi + rng, tap_i[tap]]

    for mi in range(3):
        for dt in (-1, 0, 1):
            for dw in (-1, 0, 1):
                put(mi, (dt + 1) * 3 + dw + 1, 0, (dt, -1, dw))
                put(mi, (dt + 1) * 3 + dw + 1, 1, (dt, 1, dw))
        for dw in (-1, 0, 1):
            put(mi, 9 + dw + 1, 0, (0, 0, dw))
            put(mi, 12 + dw + 1, 0, (-1, 0, dw))
            put(mi, 12 + dw + 1, 1, (1, 0, dw))
            put(mi, 15 + dw + 1, 0, (0, 0, dw))
            put(mi, 15 + dw + 1, 1, (1, 0, dw))
            put(mi, 18 + dw + 1, 0, (-1, 0, dw))
            put(mi, 18 + dw + 1, 1, (0, 0, dw))

    # exact fp16 diag tiles for v
    vd3_h = np.zeros((128, 27 * 128), np.float16)
    vd4_h = np.zeros((64, 9 * 64), np.float16)
    r64 = np.arange(64)
    for ti in range(27):
        vd3_h[rng, 128 * ti + rng] = dw_w2[384 + rng, ti].astype(np.float16)
    for si, tap in enumerate(M4_SINGLES):
        vd4_h[r64, 64 * si + r64] = \
            dw_w2[512 + r64, tap_i[tap]].astype(np.float16)
    # m4 dup-ring pair tiles: rows 0..63 diag of tap (dt,-1,dw), rows
    # 64..127 diag of tap (dt,+1,dw)
    vd4p_h = np.zeros((128, 9 * 64), np.float16)
    r64 = np.arange(64)
    for pidx, (dt, dwv) in enumerate(
            [(dt, dwv) for dt in (0, -1, 1) for dwv in (-1, 0, 1)]):
        a = tap_i[(dt, -1, dwv)]
        bb = tap_i[(dt, 1, dwv)]
        vd4p_h[r64, 64 * pidx + r64] = dw_w2[512 + r64, a].astype(np.float16)
        vd4p_h[64 + r64, 64 * pidx + r64] = \
            dw_w2[512 + r64, bb].astype(np.float16)
    # fp16 v-conv lhsT per ktile: cols 0..127 mtile3, 128..255 mtile4 dup'd
    wv16_h = np.zeros((128, 512), np.float16)
    for ki, (ko, kc) in enumerate(KTILES):
        wv16_h[:kc, 256 * ki:256 * ki + 128] = \
            qkv_w2[384:512, ko:ko + kc].T.astype(np.float16)
        wv16_h[:kc, 256 * ki + 128:256 * ki + 192] = \
            qkv_w2[512:576, ko:ko + kc].T.astype(np.float16)
        wv16_h[:kc, 256 * ki + 192:256 * ki + 256] = \
            qkv_w2[512:576, ko:ko + kc].T.astype(np.float16)

    # per-channel f32 v-diag columns for the Pool-engine taps
    vdws_h = np.zeros((128, 54), np.float32)
    vdws_h[:, 0:27] = dw_w2[384:512]
    vdws_h[:64, 27:54] = dw_w2[512:576]

    # fp8 q/k 1x1-conv weights [96, 2, 128] per qk mtile (lhsT layout:
    # W[p, j, m] = qkv_w[out=mo+m, in=p+96j])
    wq8_h = np.zeros((96, 3 * 256), FP8)
    for mi in range(3):
        for j in range(2):
            blk = qkv_w2[128 * mi:128 * (mi + 1), 96 * j:96 * (j + 1)].T
            wq8_h[:, 256 * mi + 128 * j:256 * mi + 128 * (j + 1)] = \
                blk.astype(FP8)
    projr_h = np.zeros((HD, HEADS * DIM), np.float16)
    for h in range(HEADS):
        projr_h[:, DIM * h:DIM * (h + 1)] = \
            proj_w2[:, HD * h:HD * (h + 1)].T.astype(np.float16)
    projb_h = np.zeros((128, 2), np.float32)
    projb_h[:128, 0] = np.asarray(proj_b, np.float32)[0:128]
    projb_h[:64, 1] = np.asarray(proj_b, np.float32)[128:192]
    temp_h = np.repeat(np.asarray(temperature, np.float32).reshape(HEADS, 1),
                       2, axis=1)  # [head, batch]

    in_maps = []
    for i in range(NCORES):
        # padded slab [b, t10, h10, w66], h rows 8i-1 .. 8i+9 clamped->zero
        xs = np.zeros((b_, TP, HP, XW, c_), np.float32)
        hlo, hhi = 8 * i - 1, 8 * i + 9
        slo, shi = max(0, hlo), min(h_, hhi)
        # x [b,c,t,h,w] -> [b,t,h,w,c]
        xt = x[:, :, :, slo:shi, :].transpose(0, 2, 3, 4, 1)
        xs[:, 1:9, (slo - hlo):(slo - hlo) + (shi - slo), 1:65, :] = xt
        xflat = xs.reshape(b_ * TP * HP * XW, c_)
        x16 = np.ascontiguousarray(xflat.T).astype(np.float16)
        x8_h = np.ascontiguousarray(
            xflat.T.reshape(2, 96, NPADTOK).transpose(1, 0, 2)
            .reshape(96, 2 * NPADTOK)).astype(FP8)
        in_maps.append({
            "x16": x16, "x8": x8_h, "wq8d": wq8_h, "vdws": vdws_h,
            "wv16d": wv16_h, "vdiag4p": vd4p_h,
            "qkvb": qkvb_h, "qkdiag": qkd,
            "vdiag3": vd3_h, "vdiag4": vd4_h,
            "dwb": dwb_h, "projr": projr_h, "projb": projb_h,
            "temp": temp_h,
        })
    return in_maps


def _get_runner():
    """Build once; return a persistent sharded-jit callable (the per-call
    closure in bass2jax.run_bass_via_pjrt defeats jax's jit cache)."""
    if "runner" in _CACHE:
        return _CACHE["runner"]
    import jax
    for flag, val in [("jax_compilation_cache_dir", "/tmp/jax_kernel_cache"),
                      ("jax_persistent_cache_min_compile_time_secs", 1.0),
                      ("jax_persistent_cache_min_entry_size_bytes", 0)]:
        try:
            jax.config.update(flag, val)
        except Exception:
            pass
    import jax.numpy as jnp
    from jax.sharding import Mesh, PartitionSpec
    from jax.experimental.shard_map import shard_map
    import concourse.mybir as mybir
    from concourse import bass2jax

    nc = _build()
    bass2jax.install_neuronx_cc_hook()

    partition_name = (nc.partition_id_tensor.name
                      if nc.partition_id_tensor else None)
    in_names, out_names, out_avals, zero_shapes = [], [], [], []
    for alloc in nc.m.functions[0].allocations:
        if not isinstance(alloc, mybir.MemoryLocationSet):
            continue
        name = alloc.memorylocations[0].name
        if alloc.kind == "ExternalInput":
            if name != partition_name:
                in_names.append(name)
        elif alloc.kind == "ExternalOutput":
            shape = tuple(alloc.tensor_shape)
            dtype = mybir.dt.np(alloc.dtype)
            out_names.append(name)
            out_avals.append(jax.core.ShapedArray(shape, dtype))
            zero_shapes.append((shape, dtype))
    n_params = len(in_names)
    all_names = in_names + out_names
    if partition_name is not None:
        all_names.append(partition_name)

    def _body(*args):
        operands = list(args)
        if partition_name is not None:
            operands.append(bass2jax.partition_id_tensor())
        outs = bass2jax._bass_exec_p.bind(
            *operands, out_avals=tuple(out_avals), in_names=tuple(all_names),
            out_names=tuple(out_names), lowering_input_output_aliases=(),
            sim_require_finite=True, sim_require_nnan=True, nc=nc)
        return tuple(outs)

    devices = jax.devices()[:NCORES]
    mesh = Mesh(np.asarray(devices), ("core",))
    n_outs = len(out_names)
    sharded = jax.jit(
        shard_map(_body, mesh=mesh,
                  in_specs=(PartitionSpec("core"),) * (n_params + n_outs),
                  out_specs=(PartitionSpec("core"),) * n_outs,
                  check_rep=False),
        donate_argnums=tuple(range(n_params, n_params + n_outs)),
        keep_unused=True)

    def run(in_maps):
        concat_in = [np.concatenate([in_maps[c][nm] for c in range(NCORES)],
                                    axis=0) for nm in in_names]
        concat_zeros = [np.zeros((NCORES * s[0], *s[1:]), dt)
                        for s, dt in zero_shapes]
        out_arrs = sharded(*concat_in, *concat_zeros)
        return [
            {nm: np.asarray(out_arrs[i]).reshape(NCORES, *out_avals[i].shape)[c]
             for i, nm in enumerate(out_names)}
            for c in range(NCORES)]

    _CACHE["runner"] = run
    return run


def kernel(x, qkv_w, qkv_b, dw_w, dw_b, temperature, proj_w, proj_b):
    run = _get_runner()
    in_maps = _prep_inputs(x, qkv_w, qkv_b, dw_w, dw_b, temperature,
                           proj_w, proj_b)
    results = run(in_maps)
    b_, c_, t_, h_, w_ = np.asarray(x).shape
    outf = np.empty((b_, c_, t_, h_, w_), np.float32)
    for i in range(NCORES):
        o = results[i]["out"].reshape(c_, b_, t_, H, w_)
        outf[:, :, :, 8 * i:8 * i + 8, :] = o.transpose(1, 0, 2, 3, 4)
    return outf



# revision 60
# speedup vs baseline: 1.2934x; 1.2934x over previous
"""nn_AttentionC Trainium2 kernel (8 NeuronCores, SPMD).

Sharding: h-axis (64) split into 8 chunks of 8 rows, one per core; each core's
x slab is host-padded to [b2, t10, h10, w72] fp16 tokens (conv zero-padding
baked in). Only cross-core traffic: AllReduce of per-(b,head) [48,48] q/k
gram matrices (110 KB).

Per core (PSUM fp32):
  qkv 1x1 conv on PE -> padded slabs (q/k channels quantized to fp8e4 x16,
  v channels fp16); depthwise 3x3x3:
    q/k: fp8 DoubleRow diag matmuls, two taps per matmul (taps (dt,-1,dw) and
         (dt,+1,dw) differ by 144 B in the slab = 16-aligned pair stride),
         3.6x fewer PE cycles than fp16 diag taps; softmax+normalize washes
         out the fp8 error (measured 5e-4 overall).
    v:   fp16 diag taps (fp8 on the v path fails the 2e-2 gate).
  q~,k~ transposed on PE -> [q;k] grams on PE -> AllReduce -> batched
  norm/softmax on DVE/ACT -> block-diag attn @ v on PE -> proj 1x1 conv on
  PE -> fp32 out.
"""
import numpy as np

DIM = 192
HEADS = 8
HD = DIM // HEADS  # 24
B, T, H, W = 2, 8, 8, 64  # per-core owned h rows = 8
HP, TP = 10, 10
XW = 66  # x staging row width (wpad1 + 64 + wpad1)
WP = 72  # slab row pitch: 64->72 so dh +/-1 tap pairs are 16B apart (fp8)
SLAB = HP * WP  # 720
NTOK = B * T * H * W  # 8192 owned tokens per core
NCORES = 8
C3 = 3 * DIM
NPADTOK = B * TP * HP * XW  # 13200 (x staging tokens, 66-wide rows)
ASCALE = 16.0  # fp8 slab scale
WSCALE = 64.0  # fp8 diag scale
DW_DEQ = 1.0 / (ASCALE * WSCALE)

_CACHE = {}

MTILES = [(0, 128), (128, 128), (256, 128), (384, 128), (512, 64)]
KTILES = [(0, 128), (128, 64)]
TAPS = [(dt, dh, dw) for dt in (-1, 0, 1) for dh in (-1, 0, 1)
        for dw in (-1, 0, 1)]
# fp8 DoubleRow pair plan for q/k, 21 lhsT tiles per mtile:
#  j 0..8   within-plane pairs (dt,-1,dw)+(dt,+1,dw), j = (dt+1)*3 + dw+1
#  j 9..11  center singles (0,0,dw) with zero slot B
#  j 12..14 cross-plane (-1,0,dw)+(+1,0,dw)    [interior chunks]
#  j 15..17 cross-plane (0,0,dw)+(+1,0,dw)     [t_o == 0]
#  j 18..20 cross-plane (-1,0,dw)+(0,0,dw)     [t_o == T-1]
NQK_TILES = 21  # per mtile

# ---- v depthwise engine plan ----
# m3 (v channels 384..511): 27 fp16 diag taps, split PE / DVE(ts+tt) /
# Act(mult)+DVE(add) / Act(mult)+Pool(add) / DVE(mult)+Pool(add).
# m4 (v channels 512..575, 64ch): slab ring duplicated into partitions
# 64..127 shifted by -2 rows, so (dt,-1,dw)+(dt,+1,dw) pairs become ONE
# 128-contraction matmul on PE; dh=0 taps stay singles.
# Tap lists are interleaved by dt so edge chunks (dropped dt planes)
# thin every engine roughly equally.
def _ilv(groups):
    out = []
    for i in range(max(len(g) for g in groups)):
        for g in groups:
            if i < len(g):
                out.append(g[i])
    return out


M3_TAPS = _ilv([[(dt, dh, dw) for dh in (-1, 0, 1) for dw in (-1, 0, 1)]
                for dt in (0, -1, 1)])
# per-tap engine split counts (pe, dve, act_dve, act_pool, dve_pool);
# remainder falls to dve
M3_SPLIT = (10, 10, 2, 3, 2)
M4_PAIRS = [(dt, dw) for dt in (0, -1, 1) for dw in (-1, 0, 1)]  # PE
M4_SINGLES = _ilv([[(dt, 0, dw) for dw in (-1, 0, 1)]
                   for dt in (0, -1, 1)])
M4_SPLIT = (1, 4, 0, 2, 2)
# last chunk: drain DVE early so the batch-1 softmax chain isn't stuck
# behind the tap stream
M3_SPLIT_LAST = (14, 4, 4, 3, 2)
M4_SPLIT_LAST = (3, 2, 0, 2, 2)
# qkv-conv psum eviction engines per (mtile, half) -> 'a'(Act) 'd'(DVE).
# GPSIMD/Pool cannot touch PSUM, so evictions split Act/DVE only;
# m4 entries are (base, dup) pairs.
EV_QK = ['a', 'd', 'a', 'd', 'a', 'd']   # (mi, half) for mi 0..2
EV_M3 = ['d', 'a']
EV_M4 = [('d', 'a'), ('a', 'd')]         # (base, dup) per half


def _build():
    import concourse.bacc as bacc
    import concourse.mybir as mybir
    import concourse.tile as tile
    from concourse import masks
    from concourse.ap import AP
    import bass_rust

    F32 = mybir.dt.float32
    F16 = mybir.dt.float16
    F8 = mybir.dt.float8e4
    AL = mybir.AluOpType
    AF = mybir.ActivationFunctionType
    AX = mybir.AxisListType
    DR = mybir.MatmulPerfMode.DoubleRow

    nc = bacc.Bacc("TRN2", target_bir_lowering=False, debug=False,
                   num_devices=NCORES)

    # fp16 x for the v half of the 1x1 conv (fp8 v fails the 2e-2 gate)
    x16 = nc.dram_tensor("x16", [DIM, NPADTOK], F16, kind="ExternalInput").ap()
    # fp8 copy of x, 192 channels as 2 k-tiles of 96 in the same partitions
    # (DoubleRow contraction for the q/k half of the 1x1 conv)
    x8 = nc.dram_tensor("x8", [96, 2 * NPADTOK], F8, kind="ExternalInput").ap()
    # q/k 1x1 conv weights fp8 [96, 2, 128] per qk mtile
    wq8d = nc.dram_tensor("wq8d", [96, 3 * 256], F8, kind="ExternalInput").ap()
    # fp16 v-conv weights, [kc, 256] per ktile side by side:
    # cols 0..127 m3, cols 128..255 m4 (64ch duplicated)
    wv16d = nc.dram_tensor("wv16d", [128, 512], F16,
                           kind="ExternalInput").ap()
    qkvb = nc.dram_tensor("qkvb", [128, 5], F32, kind="ExternalInput").ap()
    # per-channel f32 v-diag values for the Pool-engine taps [128, 2*27]
    vdws = nc.dram_tensor("vdws", [128, 54], F32, kind="ExternalInput").ap()
    # fp8 DoubleRow diag-pair tiles for q/k: 3 mtiles x 3 planes x 6 tiles,
    # each [128, 2, 128] fp8 (values 64*d on the diagonal)
    qkdiag = nc.dram_tensor("qkdiag", [128, 3 * NQK_TILES * 256], F8,
                            kind="ExternalInput").ap()
    # fp16 exact diag tiles for v (mtile 3: 128ch, mtile 4: 64ch)
    vdiag3 = nc.dram_tensor("vdiag3", [128, 27 * 128], F16,
                            kind="ExternalInput").ap()
    # only the 9 dh=0 taps are ever applied as m4 singles
    vdiag4 = nc.dram_tensor("vdiag4", [64, 9 * 64], F16,
                            kind="ExternalInput").ap()
    # stacked dh=-1/dh=+1 diag pairs for the dup'd m4 ring [128, 9*64]
    vdiag4p = nc.dram_tensor("vdiag4p", [128, 9 * 64], F16,
                             kind="ExternalInput").ap()
    dwb = nc.dram_tensor("dwb", [128, 5], F32, kind="ExternalInput").ap()
    # proj weights restaged per head: projr[c, 192h+m] = proj_w[m, 24h+c]
    projr = nc.dram_tensor("projr", [HD, HEADS * DIM], F16,
                           kind="ExternalInput").ap()
    projb = nc.dram_tensor("projb", [128, 2], F32, kind="ExternalInput").ap()
    temp = nc.dram_tensor("temp", [8, 2], F32, kind="ExternalInput").ap()
    out = nc.dram_tensor("out", [DIM, NTOK], F32, kind="ExternalOutput").ap()

    gram_in = nc.dram_tensor("gram_in", [16, 48, 48], F32).ap()
    gram_out = nc.dram_tensor("gram_out", [16, 48, 48], F32,
                              addr_space="Shared").ap()
    attn_dram = nc.dram_tensor("attn_dram", [16, HD, HD], F16).ap()

    with tile.TileContext(nc) as tc:
        with (
            tc.tile_pool(name="wp", bufs=1) as wp,
            tc.tile_pool(name="xp", bufs=6) as xp,
            tc.tile_pool(name="qslab", bufs=5) as slp,
            tc.tile_pool(name="qk", bufs=1) as qkpool,
            tc.tile_pool(name="ev", bufs=4) as ev,
            tc.tile_pool(name="small", bufs=1) as sp,
            tc.tile_pool(name="ps", bufs=5, space="PSUM") as psp,
            tc.tile_pool(name="psav", bufs=2, space="PSUM") as psav,
            tc.tile_pool(name="psg", bufs=1, space="PSUM") as psg,
        ):
            # ---------------- weights ----------------
            wq8 = wp.tile([96, 3 * 256], F8, tag="wq8")
            nc.sync.dma_start(out=wq8[:], in_=wq8d)
            wv16 = wp.tile([128, 512], F16, tag="wv16")
            nc.gpsimd.dma_start(out=wv16[:], in_=wv16d)
            vdws_s = wp.tile([128, 54], F32, tag="vdws")
            nc.scalar.dma_start(out=vdws_s[:], in_=vdws)
            qkvb_s = wp.tile([128, 5], F32, tag="qkvb")
            nc.sync.dma_start(out=qkvb_s[:], in_=qkvb)
            qkdiag_s = wp.tile([128, 3 * NQK_TILES * 256], F8,
                               tag="qkdiag")
            nc.scalar.dma_start(out=qkdiag_s[:], in_=qkdiag)
            vd3 = wp.tile([128, 27 * 128], F16, tag="vd3")
            nc.scalar.dma_start(out=vd3[:], in_=vdiag3)
            vd4 = wp.tile([64, 9 * 64], F16, tag="vd4")
            nc.scalar.dma_start(out=vd4[:], in_=vdiag4)
            vd4p_s = wp.tile([128, 9 * 64], F16, tag="vd4p")
            nc.gpsimd.dma_start(out=vd4p_s[:], in_=vdiag4p)
            dwb_s = wp.tile([128, 5], F32, tag="dwb")
            nc.scalar.dma_start(out=dwb_s[:], in_=dwb)
            projr_s = wp.tile([HD, HEADS * DIM], F16, tag="projr")
            nc.scalar.dma_start(out=projr_s[:], in_=projr)
            projb_s = wp.tile([128, 2], F32, tag="projb")
            nc.sync.dma_start(out=projb_s[:], in_=projb)
            temp_s = wp.tile([8, 2], F32, tag="temp")
            nc.sync.dma_start(out=temp_s[:], in_=temp)

            ident16 = wp.tile([128, 128], F16, tag="ident16")
            masks.make_identity(nc, ident16[:])
            # warm the Sqrt/Exp activation tables during the DMA-bound head
            warm = sp.tile([1, 2], F32, tag="warm")
            nc.scalar.activation(warm[:, 0:1], temp_s[0:1, 0:1], AF.Sqrt)
            nc.scalar.activation(warm[:, 1:2], temp_s[0:1, 0:1], AF.Exp)

            # dw outputs: v (192 ch) in 2 materialized tiles; q~/k~ go
            # through per-chunk ring tiles + XBAR DMA transpose into
            # qkT_all [tok128, chunk64, ch384]
            v_t = [qkpool.tile([vc, NTOK], F16, tag=f"v{i}", name=f"v{i}")
                   for i, vc in enumerate([128, 64])]
            qkT_all = qkpool.tile([128, 8, 384], F16, tag="qkT_all",
                                  name="qkT_all")
            # q/k slab rings: all 5 slots in one tensor so DoubleRow pair
            # strides may cross dt planes (slot pitch 720, 16-aligned)
            slring = [qkpool.tile([128, 5, SLAB], F8, tag=f"slr{mi}",
                                  name=f"slr{mi}") for mi in range(3)]
            # v m4 ring: partitions 0..63 = slab, 64..127 = slab shifted
            # -2 rows (so dh=-1/dh=+1 tap pairs become one PE matmul)
            ring4 = qkpool.tile([128, 5, SLAB], F16, tag="ring4",
                                name="ring4")

            def evict(engine, out_ap, in_ap, bias, scale=1.0):
                if engine == 'a':
                    nc.scalar.activation(out_ap, in_ap, AF.Identity,
                                         bias=bias, scale=scale)
                elif engine == 'd':
                    nc.vector.tensor_scalar(out_ap, in_ap, scale, bias,
                                            AL.mult, AL.add)
                else:
                    nc.gpsimd.tensor_scalar(out_ap, in_ap, scale, bias,
                                            AL.mult, AL.add)

            # ---------------- qkv conv + depthwise ----------------
            # x staging rows are 66 wide; slab rows are 72 wide (alignment
            # pad).  conv output written as [5 rows x 66] halves; q/k slab
            # cols 66..71 of each row are memset once per slab (fp8 DR
            # zero-slot singles read +16 past the data cols).
            def qkv_slab(b, t_, slabs):
                    xoff = (b * TP + t_) * HP * XW
                    # slot pitch 672 (16-aligned) holding 660 data cols
                    xx8 = xp.tile([96, 2, 672], F8, tag="x8")
                    nc.sync.dma_start(
                        out=xx8[:, :, 0:HP * XW],
                        in_=x8.rearrange("p (i n) -> p i n",
                                         i=2)[:, :, xoff:xoff + HP * XW])
                    xt = []
                    for ko, kc in KTILES:
                        xx = xp.tile([kc, HP * XW], F16, tag=f"x{ko}")
                        nc.sync.dma_start(
                            out=xx[:],
                            in_=x16[ko:ko + kc, xoff:xoff + HP * XW])
                        xt.append(xx)
                    mts = []
                    # q/k mtiles: fp8 DoubleRow conv
                    for mi in range(3):
                        sl = slring[mi][:, t_ % 5, :]
                        slr = sl.rearrange("p (h w) -> p h w", h=HP)
                        nc.gpsimd.memset(slr[:, :, XW:WP], 0.0)
                        for half in range(2):
                            ps = psp.tile([128, 512], F32, tag="mm")
                            pd = xx8.ap[0]
                            rhs = AP(tensor=xx8.tensor,
                                     offset=xx8.offset + 330 * half,
                                     ap=bass_rust.VecI64Pair(
                                         [[pd[0], pd[1]],
                                          [672, 2], [1, 330]]))
                            nc.tensor.matmul(
                                ps[:128, :330],
                                wq8[:, 256 * mi:256 * (mi + 1)]
                                .rearrange("p (i m) -> p i m", i=2),
                                rhs, start=True, stop=True, perf_mode=DR)
                            evict(EV_QK[2 * mi + half],
                                  slr[:, 5 * half:5 * (half + 1), 0:XW],
                                  ps[:128, :330].rearrange(
                                      "p (h w) -> p h w", h=5),
                                  qkvb_s[:128, mi:mi + 1], ASCALE)
                        mts.append(sl)
                    # v mtile 3: fp16 conv, 128 ch
                    sl3 = slp.tile([128, SLAB], F16, tag="sl3")
                    sl3r = sl3.rearrange("p (h w) -> p h w", h=HP)
                    for half in range(2):
                        ps = psp.tile([128, 512], F32, tag="mm")
                        for ki, (ko, kc) in enumerate(KTILES):
                            nc.tensor.matmul(
                                ps[:128, :330],
                                wv16[0:kc, 256 * ki:256 * ki + 128],
                                xt[ki][:, 330 * half:330 * (half + 1)],
                                start=(ki == 0), stop=(ki == 1))
                        evict(EV_M3[half],
                              sl3r[:, 5 * half:5 * (half + 1), 0:XW],
                              ps[:128, :330].rearrange("p (h w) -> p h w",
                                                       h=5),
                              qkvb_s[:128, 3:4])
                    mts.append(sl3)
                    # v mtile 4: fp16 conv with duplicated weights; psum
                    # partitions 64..127 hold the same 64 channels, evicted
                    # at -2 rows into the dup half of ring4
                    r4 = ring4[:, t_ % 5, :]
                    r4v = r4.rearrange("p (h w) -> p h w", h=HP)
                    for half in range(2):
                        ps = psp.tile([128, 512], F32, tag="mm")
                        for ki, (ko, kc) in enumerate(KTILES):
                            nc.tensor.matmul(
                                ps[:128, :330],
                                wv16[0:kc, 256 * ki + 128:256 * ki + 256],
                                xt[ki][:, 330 * half:330 * (half + 1)],
                                start=(ki == 0), stop=(ki == 1))
                        psv = ps[:, :330].rearrange("p (h w) -> p h w", h=5)
                        evict(EV_M4[half][0],
                              r4v[0:64, 5 * half:5 * (half + 1), 0:XW],
                              psv[0:64], qkvb_s[0:64, 4:5])
                        if half == 0:
                            evict(EV_M4[half][1], r4v[64:128, 0:3, 0:XW],
                                  psv[64:128, 2:5], qkvb_s[64:128, 4:5])
                        else:
                            evict(EV_M4[half][1], r4v[64:128, 3:8, 0:XW],
                                  psv[64:128], qkvb_s[64:128, 4:5])
                    mts.append(r4)
                    slabs[t_] = mts

            def pair_rhs(src, offA, delta):
                """[128, 2, 8, 64] view of the fp8 slab: slot i at
                offA+i*delta, then 8 rows of 64 at pitch WP."""
                pd = src.ap[0]
                return AP(tensor=src.tensor, offset=src.offset + offA,
                          ap=bass_rust.VecI64Pair(
                              [[pd[0], pd[1]], [delta, 2], [WP, 8], [1, 64]]))

            def win(src, row, dwv, mc):
                """[mc, 8, 64] window of a slab at given start row/w shift."""
                return src[:mc].rearrange(
                    "p (h w) -> p h w", h=HP)[:, row:row + 8,
                                              1 + dwv:65 + dwv]

            def gram_chunk(b, chunk):
                for c64 in range(4 * chunk, 4 * (chunk + 1)):
                    for h in range(HEADS):
                        z = qkT_all[:, c64 % 8, 48 * h:48 * (h + 1)]
                        nc.tensor.matmul(
                            gps[b][:, 48 * h:48 * (h + 1)], z, z,
                            start=(c64 == 32 * b and h == 0),
                            stop=(c64 == 32 * b + 31 and h == HEADS - 1))

            def dw_chunk(b, t_o, slabs):
                    chunk = b * T + t_o
                    co = 512 * chunk
                    planes = [dt for dt in (-1, 0, 1)
                              if not ((t_o == 0 and dt == -1) or
                                      (t_o == T - 1 and dt == 1))]
                    # q/k mtiles: fp8 DoubleRow with within-plane and
                    # cross-plane pair strides on the slab ring
                    slot = {dt: (t_o + 1 + dt) % 5 for dt in (-1, 0, 1)}
                    mms = []  # (tile j, slotA, offA, delta)
                    for dt in planes:
                        for dw in (-1, 0, 1):
                            mms.append(((dt + 1) * 3 + dw + 1, slot[dt],
                                        1 + dw, 2 * WP))
                    if len(planes) == 3:
                        for dw in (-1, 0, 1):
                            mms.append((12 + dw + 1, slot[-1], WP + 1 + dw,
                                        (slot[1] - slot[-1]) * SLAB))
                            mms.append((9 + dw + 1, slot[0], WP + 1 + dw,
                                        16))
                    elif t_o == 0:
                        for dw in (-1, 0, 1):
                            mms.append((15 + dw + 1, slot[0], WP + 1 + dw,
                                        (slot[1] - slot[0]) * SLAB))
                    else:
                        for dw in (-1, 0, 1):
                            mms.append((18 + dw + 1, slot[-1], WP + 1 + dw,
                                        (slot[0] - slot[-1]) * SLAB))
                    for mi in range(3):
                        ring = slring[mi]
                        pd = ring.ap[0]
                        ps = psp.tile([128, 512], F32, tag="mm")
                        for j, (tj, sA, offA, delta) in enumerate(mms):
                            ti = mi * NQK_TILES + tj
                            rhs = AP(tensor=ring.tensor,
                                     offset=ring.offset + sA * SLAB + offA,
                                     ap=bass_rust.VecI64Pair(
                                         [[pd[0], pd[1]], [delta, 2],
                                          [WP, 8], [1, 64]]))
                            nc.tensor.matmul(
                                ps[:128, :512],
                                qkdiag_s[:, 256 * ti:256 * (ti + 1)]
                                .rearrange("p (i m) -> p i m", i=2),
                                rhs, start=(j == 0), stop=(j == len(mms) - 1),
                                perf_mode=DR)
                        qkc = ev.tile([128, 512], F16, tag=f"qkc{mi}",
                                      name=f"qkc{mi}")
                        nc.scalar.activation(
                            qkc[:], ps[:128, :512],
                            AF.Identity, bias=dwb_s[:128, mi:mi + 1],
                            scale=DW_DEQ)
                        qdma = (nc.sync, nc.scalar, nc.sync)[mi]
                        s0 = (4 * chunk) % 8
                        qdma.dma_start_transpose(
                            out=qkT_all[:, s0:s0 + 4,
                                        128 * mi:128 * (mi + 1)],
                            in_=qkc[:])
                    # grams for the PREVIOUS chunk (its transpose DMAs have
                    # had a full chunk of time to land; PE is in-order so a
                    # not-yet-ready gram matmul would stall the dw stream)
                    if t_o > 0:
                        gram_chunk(b, chunk - 1)
                    # ---- v depthwise, engine-split per module plan ----
                    def vsplit(taps, split):
                        n_pe, n_d, n_ad, n_ap, n_dp = split
                        pe = taps[:n_pe]
                        r = taps[n_pe:]
                        # (mult_engine, add_engine) per non-PE tap
                        kinds = (['dd'] * n_d + ['ad'] * n_ad +
                                 ['ap'] * n_ap + ['dp'] * n_dp)
                        kinds += ['dd'] * (len(r) - len(kinds))
                        return pe, list(zip(kinds[:len(r)], r))

                    def vtap_nonpe(vi, mi, mc, src_of, seq,
                                   with_bias=False):
                        """Mult/add taps off the PE.  kind[0]: mult engine
                        (d=DVE a=Act); kind[1]: add chain (d=DVE p=Pool).
                        DVE and Pool accumulate in INDEPENDENT chains so a
                        slow engine never serializes the other; mults are
                        emitted first so tmps are ready when the adds run.
                        Returns a list of partial tiles to be summed.
                        with_bias: fold dw bias into the DVE chain init
                        (used when no PE taps carry it via psum)."""
                        if not seq:
                            return []

                        def dcol_of(tap):
                            ti = TAPS.index(tap)
                            return vdws_s[:mc,
                                          27 * vi + ti:27 * vi + ti + 1]

                        def win_of(tap):
                            return win(src_of(tap[0]), 1 + tap[1], tap[2],
                                       mc)

                        dch = [t for k, t in seq if k[1] == 'd']
                        pch = [t for k, t in seq if k[1] == 'p']
                        meng = {t: k[0] for k, t in seq}
                        # put a DVE-mult tap first in the DVE chain so the
                        # chain init needs no tmp (ts straight into acc)
                        dch.sort(key=lambda t: meng[t] != 'd')
                        # mults into tmp tiles (Act first, then DVE)
                        tmps = {}
                        for t in (sorted(dch + pch,
                                         key=lambda t: meng[t] != 'a')):
                            if dch and t == dch[0] and meng[t] == 'd':
                                continue
                            tmp = ev.tile(
                                [mc, 512], F16,
                                tag=f"vt{vi}{meng[t]}", name=f"tmp{vi}")
                            tmpv = tmp[:].rearrange("p (h w) -> p h w", h=8)
                            if meng[t] == 'a':
                                nc.scalar.activation(
                                    tmpv, win_of(t), AF.Identity,
                                    scale=dcol_of(t))
                            else:
                                nc.vector.tensor_scalar(
                                    tmpv, win_of(t), dcol_of(t), None,
                                    AL.mult)
                            tmps[t] = tmp
                        parts = []
                        # DVE chain: first tap multiplies straight into acc
                        if dch:
                            accd = ev.tile([mc, 512], F16, tag=f"vad{vi}",
                                           name=f"accd{vi}")
                            t0 = dch[0]
                            bias0 = (dwb_s[:mc, mi:mi + 1] if with_bias
                                     else None)
                            in0, sc = ((tmps[t0][:], 1.0) if t0 in tmps
                                       else (win_of(t0), dcol_of(t0)))
                            o = accd[:] if t0 in tmps else \
                                accd[:].rearrange("p (h w) -> p h w", h=8)
                            if bias0 is None:
                                nc.vector.tensor_scalar(o, in0, sc, None,
                                                        AL.mult)
                            else:
                                nc.vector.tensor_scalar(o, in0, sc, bias0,
                                                        AL.mult, AL.add)
                            for t in dch[1:]:
                                nc.vector.tensor_tensor(
                                    accd[:], accd[:], tmps[t][:], AL.add)
                            parts.append(accd)
                        # Pool chain: tt of the first two tmps, then adds
                        if len(pch) == 1:
                            parts.append(tmps[pch[0]])
                        elif pch:
                            accp = ev.tile([mc, 512], F16, tag=f"vap{vi}",
                                           name=f"accp{vi}")
                            nc.gpsimd.tensor_tensor(
                                accp[:], tmps[pch[0]][:], tmps[pch[1]][:],
                                AL.add)
                            for t in pch[2:]:
                                nc.gpsimd.tensor_tensor(
                                    accp[:], accp[:], tmps[t][:], AL.add)
                            parts.append(accp)
                        return parts

                    def vmerge(vi, mi, mc, parts, ps, n_pe_taps):
                        dst = v_t[vi][:, co:co + 512]
                        bias = dwb_s[:mc, mi:mi + 1]
                        if n_pe_taps and not parts:
                            nc.scalar.activation(dst, ps[:mc, :512],
                                                 AF.Identity, bias=bias)
                            return
                        if n_pe_taps:
                            tmpm = ev.tile([mc, 512], F16, tag=f"vmrg{vi}",
                                           name=f"mrg{vi}")
                            nc.scalar.activation(tmpm[:], ps[:mc, :512],
                                                 AF.Identity, bias=bias)
                            parts = parts + [tmpm]
                        while len(parts) > 2:
                            x = ev.tile([mc, 512], F16, tag=f"vmx{vi}",
                                        name=f"mx{vi}")
                            nc.vector.tensor_tensor(x[:], parts[0][:],
                                                    parts[1][:], AL.add)
                            parts = [x] + parts[2:]
                        if len(parts) == 2:
                            nc.vector.tensor_tensor(dst, parts[0][:],
                                                    parts[1][:], AL.add)
                        else:
                            nc.vector.tensor_scalar(dst, parts[0][:], 1.0,
                                                    bias, AL.mult, AL.add)

                    tvalid = lambda dt: dt in planes
                    last = (chunk == 2 * T - 1)
                    sp3 = M3_SPLIT_LAST if last else M3_SPLIT
                    sp4 = M4_SPLIT_LAST if last else M4_SPLIT
                    # m3: 27 diag taps
                    taps3 = [t for t in M3_TAPS if tvalid(t[0])]
                    pe3, seq3 = vsplit(taps3, sp3)
                    src3 = lambda dt: slabs[t_o + 1 + dt][3]
                    parts3 = vtap_nonpe(0, 3, 128, src3, seq3,
                                        with_bias=not pe3)
                    ps3 = None
                    if pe3:
                        ps3 = psp.tile([128, 512], F32, tag="mm")
                        for j, (dt, dh, dwv) in enumerate(pe3):
                            ti = TAPS.index((dt, dh, dwv))
                            nc.tensor.matmul(
                                ps3[:128, :512],
                                vd3[:, 128 * ti:128 * (ti + 1)],
                                win(src3(dt), 1 + dh, dwv, 128),
                                start=(j == 0), stop=(j == len(pe3) - 1))
                    vmerge(0, 3, 128, parts3, ps3, len(pe3))
                    # m4: PE pairs on the dup'd ring + split singles
                    pairs4 = [(dt, dw) for dt, dw in M4_PAIRS if tvalid(dt)]
                    sing4 = [t for t in M4_SINGLES if tvalid(t[0])]
                    pe4, seq4 = vsplit(sing4, sp4)
                    src4 = lambda dt: slabs[t_o + 1 + dt][4][0:64, :]
                    parts4 = vtap_nonpe(1, 4, 64, src4, seq4)
                    ps4 = psp.tile([128, 512], F32, tag="mm")
                    n4 = len(pairs4) + len(pe4)
                    pd4 = ring4.ap[0]
                    for j, (dt, dwv) in enumerate(pairs4):
                        pidx = M4_PAIRS.index((dt, dwv))
                        src = slabs[t_o + 1 + dt][4]
                        rhs = AP(tensor=src.tensor,
                                 offset=src.offset + 1 + dwv,
                                 ap=bass_rust.VecI64Pair(
                                     [[pd4[0], 128], [WP, 8], [1, 64]]))
                        nc.tensor.matmul(
                            ps4[:64, :512],
                            vd4p_s[:, 64 * pidx:64 * (pidx + 1)], rhs,
                            start=(j == 0), stop=(j == n4 - 1))
                    for j, (dt, dh, dwv) in enumerate(pe4):
                        si = M4_SINGLES.index((dt, dh, dwv))
                        nc.tensor.matmul(
                            ps4[:64, :512], vd4[:, 64 * si:64 * (si + 1)],
                            win(src4(dt), 1 + dh, dwv, 64),
                            start=(len(pairs4) + j == 0),
                            stop=(len(pairs4) + j == n4 - 1))
                    vmerge(1, 4, 64, parts4, ps4, n4)

            # per-batch norms + softmax + attn@v + proj (emitted after each
            # batch's AllReduce so batch 0's tail overlaps batch 1's dw).
            # Partition layout: (head, c) pairs in two 96-partition tiles,
            # so softmax sums fuse into the Exp via accum_out.
            def attn_batch(b):
                g8 = gram_out[8 * b:8 * (b + 1)]
                at = []
                for hh in range(2):
                    goff = g8.offset + hh * 4 * 2304
                    qq = sp.tile([96, 1], F32, tag=f"qq{hh}", name="qq")
                    nc.sync.dma_start(
                        out=qq[:],
                        in_=AP(tensor=g8.tensor, offset=goff,
                               ap=bass_rust.VecI64Pair(
                                   [[2304, 4], [49, 24]])))
                    # kk diag of head h replicated over its 24 c-partitions
                    kkr = sp.tile([96, 24], F32, tag=f"kk{hh}", name="kkr")
                    for h4 in range(4):
                        nc.scalar.dma_start(
                            out=kkr[24 * h4:24 * (h4 + 1), :],
                            in_=AP(tensor=g8.tensor,
                                   offset=goff + 2304 * h4 + 24 * 48 + 24,
                                   ap=bass_rust.VecI64Pair(
                                       [[0, 24], [49, 24]])))
                    qkf = sp.tile([96, 24], F32, tag=f"qkf{hh}", name="qkf")
                    nc.sync.dma_start(
                        out=qkf[:],
                        in_=AP(tensor=g8.tensor, offset=goff + 24,
                               ap=bass_rust.VecI64Pair(
                                   [[2304, 4], [48, 24], [1, 24]])))
                    tpr = sp.tile([96, 1], F32, tag=f"tp{hh}", name="tpr")
                    nc.gpsimd.dma_start(
                        out=tpr[:],
                        in_=AP(tensor=temp.tensor,
                               offset=temp.offset + 8 * hh + b,
                               ap=bass_rust.VecI64Pair([[2, 4], [0, 24]])))
                    # rq' = temp / max(sqrt(qq), eps) per-partition scalar
                    sq = sp.tile([96, 1], F32, tag=f"sq{hh}", name="sq")
                    nc.scalar.sqrt(sq[:], qq[:])
                    nc.vector.tensor_scalar_max(sq[:], sq[:], 1e-12)
                    rq = sp.tile([96, 1], F32, tag=f"rq{hh}", name="rq")
                    nc.vector.reciprocal(rq[:], sq[:])
                    nc.vector.tensor_tensor(rq[:], rq[:], tpr[:], AL.mult)
                    # rk = 1 / max(sqrt(kk), eps)
                    sk = sp.tile([96, 24], F32, tag=f"sk{hh}", name="sk")
                    nc.scalar.sqrt(sk[:], kkr[:])
                    nc.vector.tensor_scalar_max(sk[:], sk[:], 1e-12)
                    rk = sp.tile([96, 24], F32, tag=f"rk{hh}", name="rk")
                    nc.vector.reciprocal(rk[:], sk[:])
                    # logits, exp (+fused row-sum), renorm; |logit| <= temp
                    # so exp() is overflow-safe without max subtraction
                    a1 = sp.tile([96, 24], F32, tag=f"a1{hh}", name="a1")
                    nc.vector.tensor_scalar(a1[:], qkf[:], rq[:, 0:1], None,
                                            AL.mult)
                    nc.vector.tensor_tensor(a1[:], a1[:], rk[:], AL.mult)
                    ex = sp.tile([96, 24], F32, tag=f"ex{hh}", name="ex")
                    sm = sp.tile([96, 1], F32, tag=f"sm{hh}", name="sm")
                    nc.scalar.activation(ex[:], a1[:], AF.Exp,
                                         accum_out=sm[:])
                    rs = sp.tile([96, 1], F32, tag=f"rs{hh}", name="rs")
                    nc.vector.reciprocal(rs[:], sm[:])
                    a16 = sp.tile([96, 24], F16, tag=f"a16{hh}", name="a16")
                    nc.vector.tensor_scalar(a16[:], ex[:], rs[:, 0:1], None,
                                            AL.mult)
                    at.append(a16)

                # W2 = proj @ blockdiag(attn): per head a [24ch, 24] matmul
                # with the host-restaged projr lhsT, then transpose to get
                # the k-major lhsT for the fused (attn@v+proj) stage
                a_rhs = sp.tile([24, 8, 24], F16, tag=f"ar{b}", name="a_rhs")
                for h in range(HEADS):
                    q = nc.sync if h % 2 == 0 else nc.scalar
                    q.dma_start(
                        out=a_rhs[:, h, :],
                        in_=at[h // 4][24 * (h % 4):24 * (h % 4) + 24, :])
                w2m = []
                for mi, (mo, mc) in enumerate(KTILES):
                    pw = psav.tile([128, 512], F32, tag="av")
                    for h in range(HEADS):
                        nc.tensor.matmul(
                            pw[:mc, 24 * h:24 * (h + 1)],
                            projr_s[:, 192 * h + mo:192 * h + mo + mc],
                            a_rhs[:, h, :],
                            start=(h == 0), stop=(h == HEADS - 1))
                    wm = sp.tile([mc, DIM], F16, tag=f"w2m{b}_{mi}",
                                 name="wm")
                    nc.scalar.activation(wm[:], pw[:mc, :DIM], AF.Identity)
                    w2m.append(wm)
                w2T = []
                for ki, (ko, kc) in enumerate(KTILES):
                    wt = sp.tile([kc, DIM], F16, tag=f"w2T{b}_{ki}",
                                 name="wt")
                    for mi, (mo, mc) in enumerate(KTILES):
                        pt = psav.tile([128, 512], F32, tag="av")
                        ptv = pt.bitcast(F16)[:kc, :mc]
                        nc.tensor.transpose(
                            ptv, w2m[mi][:, ko:ko + kc], ident16[:mc, :mc])
                        nc.scalar.activation(wt[:, mo:mo + mc], ptv,
                                             AF.Identity)
                    w2T.append(wt)
                return w2T

            def av_proj(w2T, chunks):
                for chunk in chunks:
                    co = 512 * chunk
                    for mi, (mo, mc) in enumerate(KTILES):
                        ps = psav.tile([128, 512], F32, tag="av")
                        for ki in range(2):
                            nc.tensor.matmul(
                                ps[:mc, :], w2T[ki][:, mo:mo + mc],
                                v_t[ki][:, co:co + 512],
                                start=(ki == 0), stop=(ki == 1))
                        of = ev.tile([128, 512], F32, tag="of")
                        if mi == 0:
                            nc.vector.tensor_scalar(
                                of[:mc, :], ps[:mc, :],
                                projb_s[:mc, mi:mi + 1], None, AL.add)
                        else:
                            nc.scalar.activation(
                                of[:mc, :], ps[:mc, :], AF.Identity,
                                bias=projb_s[:mc, mi:mi + 1])
                        nc.sync.dma_start(out=out[mo:mo + mc, co:co + 512],
                                          in_=of[:mc, :])

            _gt = psg.tile([48, 384], F32, tag="gram", name="gram")
            gps = [_gt, _gt]
            # emission order == per-engine execution order (engines run
            # their streams in order), so nothing that waits on a collective
            # may be emitted ahead of ready work:
            #  b0 dw -> collective(0) -> b1 dw chunk 0 -> b0 softmax/W2 +
            #  av(0, 0..5) -> b1 dw rest -> collective(1) -> av(0, 5..8)
            #  (fills the collective-1 latency) -> b1 softmax/W2 -> av(1)
            w2T0 = None
            for b in range(B):
                slabs = {}
                for t_ in (1, 2, 3):
                    qkv_slab(b, t_, slabs)
                for t_o in range(T):
                    if t_o + 4 <= T:
                        qkv_slab(b, t_o + 4, slabs)
                    dw_chunk(b, t_o, slabs)
                    # batch-0 softmax + spread-out av chunks ride along
                    # batch-1's dw stream (one av chunk per dw chunk)
                    if b == 1:
                        if t_o == 1:
                            w2T0 = attn_batch(0)
                        if 1 <= t_o <= 4:
                            av_proj(w2T0, [t_o - 1])
                gram_chunk(b, b * T + T - 1)
                gs = ev.tile([48, 384], F32, tag="gs")
                nc.vector.tensor_copy(gs[:], gps[b][:])
                nc.sync.dma_start(
                    out=gram_in[8 * b:8 * (b + 1)].rearrange(
                        "g c d -> c g d"),
                    in_=gs[:].rearrange("c (g d) -> c g d", g=8))
                nc.gpsimd.collective_compute(
                    "AllReduce", AL.add,
                    replica_groups=[list(range(NCORES))],
                    ins=[gram_in[8 * b:8 * (b + 1)]],
                    outs=[gram_out[8 * b:8 * (b + 1)]])
            av_proj(w2T0, range(4, T))
            w2T1 = attn_batch(1)
            av_proj(w2T1, range(T, 2 * T))

    nc.compile()
    return nc


def _prep_inputs(x, qkv_w, qkv_b, dw_w, dw_b, temperature, proj_w, proj_b):
    """Host-side prep: per-core padded fp16 slabs + shared weights."""
    x = np.asarray(x, np.float32)
    b_, c_, t_, h_, w_ = x.shape  # 2, 192, 8, 64, 64
    qkv_w2 = np.asarray(qkv_w, np.float32).reshape(C3, DIM)
    dw_w2 = np.asarray(dw_w, np.float32).reshape(C3, 27)
    proj_w2 = np.asarray(proj_w, np.float32).reshape(DIM, DIM)
    # permute qkv channels: [q_h0, k_h0, q_h1, k_h1, ..., v] so each head's
    # (q,k) columns are adjacent after transpose (contiguous gram operands)
    perm = []
    for h in range(HEADS):
        perm.extend(range(HD * h, HD * (h + 1)))          # q_h
        perm.extend(range(DIM + HD * h, DIM + HD * (h + 1)))  # k_h
    perm.extend(range(2 * DIM, 3 * DIM))                  # v unchanged
    perm = np.array(perm)
    qkv_w2 = qkv_w2[perm]
    dw_w2 = dw_w2[perm]
    qkv_b = np.asarray(qkv_b, np.float32)[perm]
    dw_b = np.asarray(dw_b, np.float32)[perm]

    import ml_dtypes
    FP8 = ml_dtypes.float8_e4m3

    qkvb_h = np.zeros((128, 5), np.float32)
    dwb_h = np.zeros((128, 5), np.float32)
    for mi, (mo, mc) in enumerate(MTILES):
        s = ASCALE if mi < 3 else 1.0  # qk slab evicted as fp8(ASCALE*psum)
        qkvb_h[:mc, mi] = np.asarray(qkv_b, np.float32)[mo:mo + mc] * s
        dwb_h[:mc, mi] = np.asarray(dw_b, np.float32)[mo:mo + mc]
    qkvb_h[64:128, 4] = qkvb_h[0:64, 4]  # dup half of the m4 ring

    # fp8 DoubleRow diag-pair tiles for q/k (values WSCALE*d, fp8-rounded)
    tap_i = {tap: i for i, tap in enumerate(TAPS)}
    qkd = np.zeros((128, 3 * NQK_TILES * 256), FP8)
    d8 = (WSCALE * dw_w2).astype(FP8)  # [576, 27]
    rng = np.arange(128)

    def put(mi, j, slot, tap):
        base = 256 * (mi * NQK_TILES + j) + 128 * slot
        qkd[rng, base + rng] = d8[128 * mi + rng, tap_i[tap]]

    for mi in range(3):
        for dt in (-1, 0, 1):
            for dw in (-1, 0, 1):
                put(mi, (dt + 1) * 3 + dw + 1, 0, (dt, -1, dw))
                put(mi, (dt + 1) * 3 + dw + 1, 1, (dt, 1, dw))
        for dw in (-1, 0, 1):
            put(mi, 9 + dw + 1, 0, (0, 0, dw))
            put(mi, 12 + dw + 1, 0, (-1, 0, dw))
            put(mi, 12 + dw + 1, 1, (1, 0, dw))
            put(mi, 15 + dw + 1, 0, (0, 0, dw))
            put(mi, 15 + dw + 1, 1, (1, 0, dw))
            put(mi, 18 + dw + 1, 0, (-1, 0, dw))
            put(mi, 18 + dw + 1, 1, (0, 0, dw))

    # exact fp16 diag tiles for v
    vd3_h = np.zeros((128, 27 * 128), np.float16)
    vd4_h = np.zeros((64, 9 * 64), np.float16)
    r64 = np.arange(64)
    for ti in range(27):
        vd3_h[rng, 128 * ti + rng] = dw_w2[384 + rng, ti].astype(np.float16)
    for si, tap in enumerate(M4_SINGLES):
        vd4_h[r64, 64 * si + r64] = \
            dw_w2[512 + r64, tap_i[tap]].astype(np.float16)
    # m4 dup-ring pair tiles: rows 0..63 diag of tap (dt,-1,dw), rows
    # 64..127 diag of tap (dt,+1,dw)
    vd4p_h = np.zeros((128, 9 * 64), np.float16)
    r64 = np.arange(64)
    for pidx, (dt, dwv) in enumerate(
            [(dt, dwv) for dt in (0, -1, 1) for dwv in (-1, 0, 1)]):
        a = tap_i[(dt, -1, dwv)]
        bb = tap_i[(dt, 1, dwv)]
        vd4p_h[r64, 64 * pidx + r64] = dw_w2[512 + r64, a].astype(np.float16)
        vd4p_h[64 + r64, 64 * pidx + r64] = \
            dw_w2[512 + r64, bb].astype(np.float16)
    # fp16 v-conv lhsT per ktile: cols 0..127 mtile3, 128..255 mtile4 dup'd
    wv16_h = np.zeros((128, 512), np.float16)
    for ki, (ko, kc) in enumerate(KTILES):
        wv16_h[:kc, 256 * ki:256 * ki + 128] = \
            qkv_w2[384:512, ko:ko + kc].T.astype(np.float16)
        wv16_h[:kc, 256 * ki + 128:256 * ki + 192] = \
            qkv_w2[512:576, ko:ko + kc].T.astype(np.float16)
        wv16_h[:kc, 256 * ki + 192:256 * ki + 256] = \
            qkv_w2[512:576, ko:ko + kc].T.astype(np.float16)

    # per-channel f32 v-diag columns for the Pool-engine taps
    vdws_h = np.zeros((128, 54), np.float32)
    vdws_h[:, 0:27] = dw_w2[384:512]
    vdws_h[:64, 27:54] = dw_w2[512:576]

    # fp8 q/k 1x1-conv weights [96, 2, 128] per qk mtile (lhsT layout:
    # W[p, j, m] = qkv_w[out=mo+m, in=p+96j])
    wq8_h = np.zeros((96, 3 * 256), FP8)
    for mi in range(3):
        for j in range(2):
            blk = qkv_w2[128 * mi:128 * (mi + 1), 96 * j:96 * (j + 1)].T
            wq8_h[:, 256 * mi + 128 * j:256 * mi + 128 * (j + 1)] = \
                blk.astype(FP8)
    projr_h = np.zeros((HD, HEADS * DIM), np.float16)
    for h in range(HEADS):
        projr_h[:, DIM * h:DIM * (h + 1)] = \
            proj_w2[:, HD * h:HD * (h + 1)].T.astype(np.float16)
    projb_h = np.zeros((128, 2), np.float32)
    projb_h[:128, 0] = np.asarray(proj_b, np.float32)[0:128]
    projb_h[:64, 1] = np.asarray(proj_b, np.float32)[128:192]
    temp_h = np.repeat(np.asarray(temperature, np.float32).reshape(HEADS, 1),
                       2, axis=1)  # [head, batch]

    in_maps = []
    for i in range(NCORES):
        # padded slab [b, t10, h10, w66], h rows 8i-1 .. 8i+9 clamped->zero
        xs = np.zeros((b_, TP, HP, XW, c_), np.float32)
        hlo, hhi = 8 * i - 1, 8 * i + 9
        slo, shi = max(0, hlo), min(h_, hhi)
        # x [b,c,t,h,w] -> [b,t,h,w,c]
        xt = x[:, :, :, slo:shi, :].transpose(0, 2, 3, 4, 1)
        xs[:, 1:9, (slo - hlo):(slo - hlo) + (shi - slo), 1:65, :] = xt
        xflat = xs.reshape(b_ * TP * HP * XW, c_)
        x16 = np.ascontiguousarray(xflat.T).astype(np.float16)
        x8_h = np.ascontiguousarray(
            xflat.T.reshape(2, 96, NPADTOK).transpose(1, 0, 2)
            .reshape(96, 2 * NPADTOK)).astype(FP8)
        in_maps.append({
            "x16": x16, "x8": x8_h, "wq8d": wq8_h, "vdws": vdws_h,
            "wv16d": wv16_h, "vdiag4p": vd4p_h,
            "qkvb": qkvb_h, "qkdiag": qkd,
            "vdiag3": vd3_h, "vdiag4": vd4_h,
            "dwb": dwb_h, "projr": projr_h, "projb": projb_h,
            "temp": temp_h,
        })
    return in_maps


def _get_runner():
    """Build once; return a persistent sharded-jit callable (the per-call
    closure in bass2jax.run_bass_via_pjrt defeats jax's jit cache)."""
    if "runner" in _CACHE:
        return _CACHE["runner"]
    import jax
    for flag, val in [("jax_compilation_cache_dir", "/tmp/jax_kernel_cache"),
                      ("jax_persistent_cache_min_compile_time_secs", 1.0),
                      ("jax_persistent_cache_min_entry_size_bytes", 0)]:
        try:
            jax.config.update(flag, val)
        except Exception:
            pass
    import jax.numpy as jnp
    from jax.sharding import Mesh, PartitionSpec
    from jax.experimental.shard_map import shard_map
    import concourse.mybir as mybir
    from concourse import bass2jax

    nc = _build()
    bass2jax.install_neuronx_cc_hook()

    partition_name = (nc.partition_id_tensor.name
                      if nc.partition_id_tensor else None)
    in_names, out_names, out_avals, zero_shapes = [], [], [], []
    for alloc in nc.m.functions[0].allocations:
        if not isinstance(alloc, mybir.MemoryLocationSet):
            continue
        name = alloc.memorylocations[0].name
        if alloc.kind == "ExternalInput":
            if name != partition_name:
                in_names.append(name)
        elif alloc.kind == "ExternalOutput":
            shape = tuple(alloc.tensor_shape)
            dtype = mybir.dt.np(alloc.dtype)
            out_names.append(name)
            out_avals.append(jax.core.ShapedArray(shape, dtype))
            zero_shapes.append((shape, dtype))
    n_params = len(in_names)
    all_names = in_names + out_names
    if partition_name is not None:
        all_names.append(partition_name)

    def _body(*args):
        operands = list(args)
        if partition_name is not None:
            operands.append(bass2jax.partition_id_tensor())
        outs = bass2jax._bass_exec_p.bind(
            *operands, out_avals=tuple(out_avals), in_names=tuple(all_names),
            out_names=tuple(out_names), lowering_input_output_aliases=(),
            sim_require_finite=True, sim_require_nnan=True, nc=nc)
        return tuple(outs)

    devices = jax.devices()[:NCORES]
    mesh = Mesh(np.asarray(devices), ("core",))
    n_outs = len(out_names)
    sharded = jax.jit(
        shard_map(_body, mesh=mesh,
                  in_specs=(PartitionSpec("core"),) * (n_params + n_outs),
                  out_specs=(PartitionSpec("core"),) * n_outs,
                  check_rep=False),
        donate_argnums=tuple(range(n_params, n_params + n_outs)),
        keep_unused=True)

    def run(in_maps):
        concat_in = [np.concatenate([in_maps[c][nm] for c in range(NCORES)],
                                    axis=0) for nm in in_names]
        concat_zeros = [np.zeros((NCORES * s[0], *s[1:]), dt)
                        for s, dt in zero_shapes]
        out_arrs = sharded(*concat_in, *concat_zeros)
        return [
            {nm: np.asarray(out_arrs[i]).reshape(NCORES, *out_avals[i].shape)[c]
             for i, nm in enumerate(out_names)}
            for c in range(NCORES)]

    _CACHE["runner"] = run
    return run


def kernel(x, qkv_w, qkv_b, dw_w, dw_b, temperature, proj_w, proj_b):
    run = _get_runner()
    in_maps = _prep_inputs(x, qkv_w, qkv_b, dw_w, dw_b, temperature,
                           proj_w, proj_b)
    results = run(in_maps)
    b_, c_, t_, h_, w_ = np.asarray(x).shape
    outf = np.empty((b_, c_, t_, h_, w_), np.float32)
    for i in range(NCORES):
        o = results[i]["out"].reshape(c_, b_, t_, H, w_)
        outf[:, :, :, 8 * i:8 * i + 8, :] = o.transpose(1, 0, 2, 3, 4)
    return outf



# revision 90
# speedup vs baseline: 1.3334x; 1.0309x over previous
"""nn_AttentionC Trainium2 kernel (8 NeuronCores, SPMD).

Sharding: h-axis (64) split into 8 chunks of 8 rows, one per core; each core's
x slab is host-padded to [b2, t10, h10, w72] fp16 tokens (conv zero-padding
baked in). Only cross-core traffic: AllReduce of per-(b,head) [48,48] q/k
gram matrices (110 KB).

Per core (PSUM fp32):
  qkv 1x1 conv on PE -> padded slabs (q/k channels quantized to fp8e4 x16,
  v channels fp16); depthwise 3x3x3:
    q/k: fp8 DoubleRow diag matmuls, two taps per matmul (taps (dt,-1,dw) and
         (dt,+1,dw) differ by 144 B in the slab = 16-aligned pair stride),
         3.6x fewer PE cycles than fp16 diag taps; softmax+normalize washes
         out the fp8 error (measured 5e-4 overall).
    v:   fp16 diag taps (fp8 on the v path fails the 2e-2 gate).
  q~,k~ transposed on PE -> [q;k] grams on PE -> AllReduce -> batched
  norm/softmax on DVE/ACT -> block-diag attn @ v on PE -> proj 1x1 conv on
  PE -> fp32 out.
"""
import numpy as np

DIM = 192
HEADS = 8
HD = DIM // HEADS  # 24
B, T, H, W = 2, 8, 8, 64  # per-core owned h rows = 8
HP, TP = 10, 10
XW = 66  # x staging row width (wpad1 + 64 + wpad1)
WP = 72  # slab row pitch: 64->72 so dh +/-1 tap pairs are 16B apart (fp8)
SLAB = HP * WP  # 720
NTOK = B * T * H * W  # 8192 owned tokens per core
NCORES = 8
C3 = 3 * DIM
NPADTOK = B * TP * HP * XW  # 13200 (x staging tokens, 66-wide rows)
ASCALE = 16.0  # fp8 slab scale
WSCALE = 64.0  # fp8 diag scale
DW_DEQ = 1.0 / (ASCALE * WSCALE)

_CACHE = {}

MTILES = [(0, 128), (128, 128), (256, 128), (384, 128), (512, 64)]
KTILES = [(0, 128), (128, 64)]
TAPS = [(dt, dh, dw) for dt in (-1, 0, 1) for dh in (-1, 0, 1)
        for dw in (-1, 0, 1)]
# fp8 DoubleRow pair plan for q/k, 21 lhsT tiles per mtile:
#  j 0..8   within-plane pairs (dt,-1,dw)+(dt,+1,dw), j = (dt+1)*3 + dw+1
#  j 9..11  center singles (0,0,dw) with zero slot B
#  j 12..14 cross-plane (-1,0,dw)+(+1,0,dw)    [interior chunks]
#  j 15..17 cross-plane (0,0,dw)+(+1,0,dw)     [t_o == 0]
#  j 18..20 cross-plane (-1,0,dw)+(0,0,dw)     [t_o == T-1]
NQK_TILES = 21  # per mtile

# ---- v depthwise engine plan ----
# m3 (v channels 384..511): 27 fp16 diag taps, split PE / DVE(ts+tt) /
# Act(mult)+DVE(add) / Act(mult)+Pool(add) / DVE(mult)+Pool(add).
# m4 (v channels 512..575, 64ch): slab ring duplicated into partitions
# 64..127 shifted by -2 rows, so (dt,-1,dw)+(dt,+1,dw) pairs become ONE
# 128-contraction matmul on PE; dh=0 taps stay singles.
# Tap lists are interleaved by dt so edge chunks (dropped dt planes)
# thin every engine roughly equally.
def _ilv(groups):
    out = []
    for i in range(max(len(g) for g in groups)):
        for g in groups:
            if i < len(g):
                out.append(g[i])
    return out


M3_TAPS = _ilv([[(dt, dh, dw) for dh in (-1, 0, 1) for dw in (-1, 0, 1)]
                for dt in (0, -1, 1)])
# per-tap engine split counts (pe, dve, act_dve, act_pool, dve_pool);
# remainder falls to dve
M3_SPLIT = (10, 10, 2, 3, 2)
M4_PAIRS = [(dt, dw) for dt in (0, -1, 1) for dw in (-1, 0, 1)]  # PE
M4_SINGLES = _ilv([[(dt, 0, dw) for dw in (-1, 0, 1)]
                   for dt in (0, -1, 1)])
M4_SPLIT = (1, 4, 0, 2, 2)
# last chunk: drain DVE early so the batch-1 softmax chain isn't stuck
# behind the tap stream
M3_SPLIT_LAST = (14, 4, 4, 3, 2)
M4_SPLIT_LAST = (3, 2, 0, 2, 2)
# qkv-conv psum eviction engines per (mtile, half) -> 'a'(Act) 'd'(DVE).
# GPSIMD/Pool cannot touch PSUM, so evictions split Act/DVE only;
# m4 entries are (base, dup) pairs.
EV_QK = ['a', 'd', 'a', 'd', 'a', 'd']   # (mi, half) for mi 0..2
EV_M3 = ['d', 'a']
EV_M4 = [('d', 'a'), ('a', 'd')]         # (base, dup) per half


def _build():
    import concourse.bacc as bacc
    import concourse.mybir as mybir
    import concourse.tile as tile
    from concourse import masks
    from concourse.ap import AP
    import bass_rust

    # this kernel only uses identity/ln/exp activations, all present in
    # the natural_log_exp_and_others table set; restricting the table list
    # pins ONE set so no mid-stream LoadActFuncSet reloads are emitted
    if not getattr(bacc, "_lnexp_only", False):
        _orig_tables = bacc.get_activation_tables

        def _lnexp_tables(arch):
            t = _orig_tables(arch)
            keep = {k: v for k, v in t.items()
                    if k == "natural_log_exp_and_others"}
            return keep or t

        bacc.get_activation_tables = _lnexp_tables
        bacc._lnexp_only = True

    F32 = mybir.dt.float32
    F16 = mybir.dt.float16
    F8 = mybir.dt.float8e4
    AL = mybir.AluOpType
    AF = mybir.ActivationFunctionType
    AX = mybir.AxisListType
    DR = mybir.MatmulPerfMode.DoubleRow

    nc = bacc.Bacc("TRN2", target_bir_lowering=False, debug=False,
                   num_devices=NCORES)

    # fp16 x for the v half of the 1x1 conv (fp8 v fails the 2e-2 gate)
    x16 = nc.dram_tensor("x16", [DIM, NPADTOK], F16, kind="ExternalInput").ap()
    # fp8 copy of x, 192 channels as 2 k-tiles of 96 in the same partitions
    # (DoubleRow contraction for the q/k half of the 1x1 conv)
    x8 = nc.dram_tensor("x8", [96, 2 * NPADTOK], F8, kind="ExternalInput").ap()
    # q/k 1x1 conv weights fp8 [96, 2, 128] per qk mtile
    wq8d = nc.dram_tensor("wq8d", [96, 3 * 256], F8, kind="ExternalInput").ap()
    # fp16 v-conv weights, [kc, 256] per ktile side by side:
    # cols 0..127 m3, cols 128..255 m4 (64ch duplicated)
    wv16d = nc.dram_tensor("wv16d", [128, 512], F16,
                           kind="ExternalInput").ap()
    qkvb = nc.dram_tensor("qkvb", [128, 5], F32, kind="ExternalInput").ap()
    # per-channel f32 v-diag values for the Pool-engine taps [128, 2*27]
    vdws = nc.dram_tensor("vdws", [128, 54], F32, kind="ExternalInput").ap()
    # fp8 DoubleRow diag-pair tiles for q/k: 3 mtiles x 3 planes x 6 tiles,
    # each [128, 2, 128] fp8 (values 64*d on the diagonal)
    qkdiag = nc.dram_tensor("qkdiag", [128, 3 * NQK_TILES * 256], F8,
                            kind="ExternalInput").ap()
    # fp16 exact diag tiles for v (mtile 3: 128ch, mtile 4: 64ch)
    vdiag3 = nc.dram_tensor("vdiag3", [128, 27 * 128], F16,
                            kind="ExternalInput").ap()
    # only the 9 dh=0 taps are ever applied as m4 singles
    vdiag4 = nc.dram_tensor("vdiag4", [64, 9 * 64], F16,
                            kind="ExternalInput").ap()
    # stacked dh=-1/dh=+1 diag pairs for the dup'd m4 ring [128, 9*64]
    vdiag4p = nc.dram_tensor("vdiag4p", [128, 9 * 64], F16,
                             kind="ExternalInput").ap()
    dwb = nc.dram_tensor("dwb", [128, 5], F32, kind="ExternalInput").ap()
    # proj weights restaged per head: projr[c, 192h+m] = proj_w[m, 24h+c]
    projr = nc.dram_tensor("projr", [HD, HEADS * DIM], F16,
                           kind="ExternalInput").ap()
    projb = nc.dram_tensor("projb", [128, 2], F32, kind="ExternalInput").ap()
    # cols 0-1: temperature per (head, batch); cols 2-3: ln(temperature)
    temp = nc.dram_tensor("temp", [8, 4], F32, kind="ExternalInput").ap()
    # head-broadcast selector: hsel[h, 96*hh + 24*(h-4*hh) + c] = 1
    hsel = nc.dram_tensor("hsel", [8, 192], F16, kind="ExternalInput").ap()
    out = nc.dram_tensor("out", [DIM, NTOK], F32, kind="ExternalOutput").ap()

    gram_in = nc.dram_tensor("gram_in", [16, 48, 48], F32).ap()
    gram_out = nc.dram_tensor("gram_out", [16, 48, 48], F32,
                              addr_space="Shared").ap()
    attn_dram = nc.dram_tensor("attn_dram", [16, HD, HD], F16).ap()

    with tile.TileContext(nc) as tc:
        with (
            tc.tile_pool(name="wp", bufs=1) as wp,
            tc.tile_pool(name="xp", bufs=6) as xp,
            tc.tile_pool(name="qslab", bufs=6) as slp,
            tc.tile_pool(name="qk", bufs=1) as qkpool,
            tc.tile_pool(name="ev", bufs=3) as ev,
            tc.tile_pool(name="op", bufs=2) as op,
            tc.tile_pool(name="small", bufs=1) as sp,
            tc.tile_pool(name="ps", bufs=5, space="PSUM") as psp,
            tc.tile_pool(name="psav", bufs=2, space="PSUM") as psav,
            tc.tile_pool(name="psg", bufs=1, space="PSUM") as psg,
        ):
            # ---------------- weights ----------------
            wq8 = wp.tile([96, 3 * 256], F8, tag="wq8")
            nc.sync.dma_start(out=wq8[:], in_=wq8d)
            wv16 = wp.tile([128, 512], F16, tag="wv16")
            nc.gpsimd.dma_start(out=wv16[:], in_=wv16d)
            qkvb_s = wp.tile([128, 5], F32, tag="qkvb")
            nc.sync.dma_start(out=qkvb_s[:], in_=qkvb)
            # weights not needed by the first conv: DMAs deferred past the
            # first slab's x loads (HWDGE generates one descriptor ~630ns,
            # shared by every queue)
            vdws_s = wp.tile([128, 54], F32, tag="vdws")
            qkdiag_s = wp.tile([128, 3 * NQK_TILES * 256], F8,
                               tag="qkdiag")
            vd3 = wp.tile([128, 27 * 128], F16, tag="vd3")
            vd4 = wp.tile([64, 9 * 64], F16, tag="vd4")
            vd4p_s = wp.tile([128, 9 * 64], F16, tag="vd4p")
            dwb_s = wp.tile([128, 5], F32, tag="dwb")
            projr_s = wp.tile([HD, HEADS * DIM], F16, tag="projr")
            projb_s = wp.tile([128, 2], F32, tag="projb")
            hsel_s = wp.tile([8, 192], F16, tag="hsel")
            tln = [[wp.tile([96, 1], F32, tag=f"tln{b}{hh}",
                            name=f"tln{b}{hh}")
                    for hh in range(2)] for b in range(2)]

            def load_late_weights():
                nc.scalar.dma_start(out=vdws_s[:], in_=vdws)
                nc.scalar.dma_start(out=qkdiag_s[:], in_=qkdiag)
                nc.scalar.dma_start(out=vd3[:], in_=vdiag3)
                nc.scalar.dma_start(out=vd4[:], in_=vdiag4)
                nc.gpsimd.dma_start(out=vd4p_s[:], in_=vdiag4p)
                nc.scalar.dma_start(out=dwb_s[:], in_=dwb)
                nc.scalar.dma_start(out=projr_s[:], in_=projr)
                nc.sync.dma_start(out=projb_s[:], in_=projb)
                nc.gpsimd.dma_start(out=hsel_s[:], in_=hsel)
                for b in range(2):
                    for hh in range(2):
                        nc.gpsimd.dma_start(
                            out=tln[b][hh][:],
                            in_=AP(tensor=temp.tensor,
                                   offset=temp.offset + 16 * hh + 2 + b,
                                   ap=bass_rust.VecI64Pair(
                                       [[4, 4], [0, 24]])))

            ident16 = wp.tile([128, 128], F16, tag="ident16")
            masks.make_identity(nc, ident16[:])
            # -6 ln 2: exp-bias compensating the kk 2^-12 f16 prescale
            kbias = wp.tile([128, 1], F32, tag="kbias")
            nc.vector.memset(kbias[:], -6.0 * 0.6931471805599453)
            # the whole kernel only ever needs the ln/exp/identity table
            # set; warm it once during the DMA-bound head
            warm = sp.tile([1, 2], F32, tag="warm")
            nc.scalar.activation(warm[:, 0:1], ident16[0:1, 0:1], AF.Ln)
            nc.scalar.activation(warm[:, 1:2], ident16[0:1, 0:1], AF.Exp)

            # dw outputs: v (192 ch) in 2 materialized tiles; q~/k~ go
            # through per-chunk ring tiles + XBAR DMA transpose into
            # qkT_all [tok128, chunk64, ch384]
            v_t = [qkpool.tile([vc, NTOK], F16, tag=f"v{i}", name=f"v{i}")
                   for i, vc in enumerate([128, 64])]
            qkT_all = qkpool.tile([128, 8, 384], F16, tag="qkT_all",
                                  name="qkT_all")
            # q/k slab rings: all 5 slots in one tensor so DoubleRow pair
            # strides may cross dt planes (slot pitch 720, 16-aligned)
            slring = [qkpool.tile([128, 6, SLAB], F8, tag=f"slr{mi}",
                                  name=f"slr{mi}") for mi in range(3)]
            # v m4 ring: partitions 0..63 = slab, 64..127 = slab shifted
            # -2 rows (so dh=-1/dh=+1 tap pairs become one PE matmul)
            ring4 = qkpool.tile([128, 6, SLAB], F16, tag="ring4",
                                name="ring4")

            def evict(engine, out_ap, in_ap, bias, scale=1.0):
                if engine == 'a':
                    nc.scalar.activation(out_ap, in_ap, AF.Identity,
                                         bias=bias, scale=scale)
                elif engine == 'd':
                    nc.vector.tensor_scalar(out_ap, in_ap, scale, bias,
                                            AL.mult, AL.add)
                else:
                    nc.gpsimd.tensor_scalar(out_ap, in_ap, scale, bias,
                                            AL.mult, AL.add)

            # ---------------- qkv conv + depthwise ----------------
            # x staging rows are 66 wide; slab rows are 72 wide (alignment
            # pad).  conv output written as [5 rows x 66] halves; q/k slab
            # cols 66..71 of each row are memset once per slab (fp8 DR
            # zero-slot singles read +16 past the data cols).
            def qkv_slab(b, t_, slabs):
                    xoff = (b * TP + t_) * HP * XW
                    # slot pitch 672 (16-aligned) holding 660 data cols
                    xx8 = xp.tile([96, 2, 672], F8, tag="x8")
                    nc.sync.dma_start(
                        out=xx8[:, :, 0:HP * XW],
                        in_=x8.rearrange("p (i n) -> p i n",
                                         i=2)[:, :, xoff:xoff + HP * XW])
                    xt = []
                    for ko, kc in KTILES:
                        xx = xp.tile([kc, HP * XW], F16, tag=f"x{ko}")
                        nc.sync.dma_start(
                            out=xx[:],
                            in_=x16[ko:ko + kc, xoff:xoff + HP * XW])
                        xt.append(xx)
                    mts = []
                    # q/k mtiles: fp8 DoubleRow conv
                    for mi in range(3):
                        sl = slring[mi][:, t_ % 6, :]
                        slr = sl.rearrange("p (h w) -> p h w", h=HP)
                        nc.gpsimd.memset(slr[:, :, XW:WP], 0.0)
                        for half in range(2):
                            ps = psp.tile([128, 512], F32, tag="mm")
                            pd = xx8.ap[0]
                            rhs = AP(tensor=xx8.tensor,
                                     offset=xx8.offset + 330 * half,
                                     ap=bass_rust.VecI64Pair(
                                         [[pd[0], pd[1]],
                                          [672, 2], [1, 330]]))
                            nc.tensor.matmul(
                                ps[:128, :330],
                                wq8[:, 256 * mi:256 * (mi + 1)]
                                .rearrange("p (i m) -> p i m", i=2),
                                rhs, start=True, stop=True, perf_mode=DR)
                            evict(EV_QK[2 * mi + half],
                                  slr[:, 5 * half:5 * (half + 1), 0:XW],
                                  ps[:128, :330].rearrange(
                                      "p (h w) -> p h w", h=5),
                                  qkvb_s[:128, mi:mi + 1], ASCALE)
                        mts.append(sl)
                    # v mtile 3: fp16 conv, 128 ch
                    sl3 = slp.tile([128, SLAB], F16, tag="sl3")
                    sl3r = sl3.rearrange("p (h w) -> p h w", h=HP)
                    for half in range(2):
                        ps = psp.tile([128, 512], F32, tag="mm")
                        for ki, (ko, kc) in enumerate(KTILES):
                            nc.tensor.matmul(
                                ps[:128, :330],
                                wv16[0:kc, 256 * ki:256 * ki + 128],
                                xt[ki][:, 330 * half:330 * (half + 1)],
                                start=(ki == 0), stop=(ki == 1))
                        evict(EV_M3[half],
                              sl3r[:, 5 * half:5 * (half + 1), 0:XW],
                              ps[:128, :330].rearrange("p (h w) -> p h w",
                                                       h=5),
                              qkvb_s[:128, 3:4])
                    mts.append(sl3)
                    # v mtile 4: fp16 conv with duplicated weights; psum
                    # partitions 64..127 hold the same 64 channels, evicted
                    # at -2 rows into the dup half of ring4
                    r4 = ring4[:, t_ % 6, :]
                    r4v = r4.rearrange("p (h w) -> p h w", h=HP)
                    for half in range(2):
                        ps = psp.tile([128, 512], F32, tag="mm")
                        for ki, (ko, kc) in enumerate(KTILES):
                            nc.tensor.matmul(
                                ps[:128, :330],
                                wv16[0:kc, 256 * ki + 128:256 * ki + 256],
                                xt[ki][:, 330 * half:330 * (half + 1)],
                                start=(ki == 0), stop=(ki == 1))
                        psv = ps[:, :330].rearrange("p (h w) -> p h w", h=5)
                        evict(EV_M4[half][0],
                              r4v[0:64, 5 * half:5 * (half + 1), 0:XW],
                              psv[0:64], qkvb_s[0:64, 4:5])
                        if half == 0:
                            evict(EV_M4[half][1], r4v[64:128, 0:3, 0:XW],
                                  psv[64:128, 2:5], qkvb_s[64:128, 4:5])
                        else:
                            evict(EV_M4[half][1], r4v[64:128, 3:8, 0:XW],
                                  psv[64:128], qkvb_s[64:128, 4:5])
                    mts.append(r4)
                    slabs[t_] = mts

            def pair_rhs(src, offA, delta):
                """[128, 2, 8, 64] view of the fp8 slab: slot i at
                offA+i*delta, then 8 rows of 64 at pitch WP."""
                pd = src.ap[0]
                return AP(tensor=src.tensor, offset=src.offset + offA,
                          ap=bass_rust.VecI64Pair(
                              [[pd[0], pd[1]], [delta, 2], [WP, 8], [1, 64]]))

            def win(src, row, dwv, mc):
                """[mc, 8, 64] window of a slab at given start row/w shift."""
                return src[:mc].rearrange(
                    "p (h w) -> p h w", h=HP)[:, row:row + 8,
                                              1 + dwv:65 + dwv]

            def gram_chunk(b, chunk):
                for c64 in range(4 * chunk, 4 * (chunk + 1)):
                    for h in range(HEADS):
                        z = qkT_all[:, c64 % 8, 48 * h:48 * (h + 1)]
                        nc.tensor.matmul(
                            gps[b][:, 48 * h:48 * (h + 1)], z, z,
                            start=(c64 == 32 * b and h == 0),
                            stop=(c64 == 32 * b + 31 and h == HEADS - 1))

            def dw_chunk(b, t_o, slabs):
                    chunk = b * T + t_o
                    co = 512 * chunk
                    planes = [dt for dt in (-1, 0, 1)
                              if not ((t_o == 0 and dt == -1) or
                                      (t_o == T - 1 and dt == 1))]
                    # q/k mtiles: fp8 DoubleRow with within-plane and
                    # cross-plane pair strides on the slab ring
                    slot = {dt: (t_o + 1 + dt) % 6 for dt in (-1, 0, 1)}
                    mms = []  # (tile j, slotA, offA, delta)
                    for dt in planes:
                        for dw in (-1, 0, 1):
                            mms.append(((dt + 1) * 3 + dw + 1, slot[dt],
                                        1 + dw, 2 * WP))
                    if len(planes) == 3:
                        for dw in (-1, 0, 1):
                            mms.append((12 + dw + 1, slot[-1], WP + 1 + dw,
                                        (slot[1] - slot[-1]) * SLAB))
                            mms.append((9 + dw + 1, slot[0], WP + 1 + dw,
                                        16))
                    elif t_o == 0:
                        for dw in (-1, 0, 1):
                            mms.append((15 + dw + 1, slot[0], WP + 1 + dw,
                                        (slot[1] - slot[0]) * SLAB))
                    else:
                        for dw in (-1, 0, 1):
                            mms.append((18 + dw + 1, slot[-1], WP + 1 + dw,
                                        (slot[0] - slot[-1]) * SLAB))
                    for mi in range(3):
                        ring = slring[mi]
                        pd = ring.ap[0]
                        ps = psp.tile([128, 512], F32, tag="mm")
                        for j, (tj, sA, offA, delta) in enumerate(mms):
                            ti = mi * NQK_TILES + tj
                            rhs = AP(tensor=ring.tensor,
                                     offset=ring.offset + sA * SLAB + offA,
                                     ap=bass_rust.VecI64Pair(
                                         [[pd[0], pd[1]], [delta, 2],
                                          [WP, 8], [1, 64]]))
                            nc.tensor.matmul(
                                ps[:128, :512],
                                qkdiag_s[:, 256 * ti:256 * (ti + 1)]
                                .rearrange("p (i m) -> p i m", i=2),
                                rhs, start=(j == 0), stop=(j == len(mms) - 1),
                                perf_mode=DR)
                        qkc = ev.tile([128, 512], F16, tag=f"qkc{mi}",
                                      name=f"qkc{mi}")
                        nc.scalar.activation(
                            qkc[:], ps[:128, :512],
                            AF.Identity, bias=dwb_s[:128, mi:mi + 1],
                            scale=DW_DEQ)
                        qdma = (nc.sync, nc.scalar, nc.sync)[mi]
                        s0 = (4 * chunk) % 8
                        qdma.dma_start_transpose(
                            out=qkT_all[:, s0:s0 + 4,
                                        128 * mi:128 * (mi + 1)],
                            in_=qkc[:])
                    # grams for the PREVIOUS chunk (its transpose DMAs have
                    # had a full chunk of time to land; PE is in-order so a
                    # not-yet-ready gram matmul would stall the dw stream)
                    if t_o > 0:
                        gram_chunk(b, chunk - 1)
                    # ---- v depthwise, engine-split per module plan ----
                    def vsplit(taps, split):
                        n_pe, n_d, n_ad, n_ap, n_dp = split
                        pe = taps[:n_pe]
                        r = taps[n_pe:]
                        # (mult_engine, add_engine) per non-PE tap
                        kinds = (['dd'] * n_d + ['ad'] * n_ad +
                                 ['ap'] * n_ap + ['dp'] * n_dp)
                        kinds += ['dd'] * (len(r) - len(kinds))
                        return pe, list(zip(kinds[:len(r)], r))

                    def vtap_nonpe(vi, mi, mc, src_of, seq,
                                   with_bias=False):
                        """Mult/add taps off the PE.  kind[0]: mult engine
                        (d=DVE a=Act); kind[1]: add chain (d=DVE p=Pool).
                        DVE and Pool accumulate in INDEPENDENT chains so a
                        slow engine never serializes the other; mults are
                        emitted first so tmps are ready when the adds run.
                        Returns a list of partial tiles to be summed.
                        with_bias: fold dw bias into the DVE chain init
                        (used when no PE taps carry it via psum)."""
                        if not seq:
                            return []

                        def dcol_of(tap):
                            ti = TAPS.index(tap)
                            return vdws_s[:mc,
                                          27 * vi + ti:27 * vi + ti + 1]

                        def win_of(tap):
                            return win(src_of(tap[0]), 1 + tap[1], tap[2],
                                       mc)

                        dch = [t for k, t in seq if k[1] == 'd']
                        pch = [t for k, t in seq if k[1] == 'p']
                        meng = {t: k[0] for k, t in seq}
                        # put a DVE-mult tap first in the DVE chain so the
                        # chain init needs no tmp (ts straight into acc)
                        dch.sort(key=lambda t: meng[t] != 'd')
                        # mults into tmp tiles (Act first, then DVE)
                        tmps = {}
                        for t in (sorted(dch + pch,
                                         key=lambda t: meng[t] != 'a')):
                            if dch and t == dch[0] and meng[t] == 'd':
                                continue
                            tmp = ev.tile(
                                [mc, 512], F16,
                                tag=f"vt{vi}{meng[t]}", name=f"tmp{vi}")
                            tmpv = tmp[:].rearrange("p (h w) -> p h w", h=8)
                            if meng[t] == 'a':
                                nc.scalar.activation(
                                    tmpv, win_of(t), AF.Identity,
                                    scale=dcol_of(t))
                            else:
                                nc.vector.tensor_scalar(
                                    tmpv, win_of(t), dcol_of(t), None,
                                    AL.mult)
                            tmps[t] = tmp
                        parts = []
                        # DVE chain: first tap multiplies straight into acc
                        if dch:
                            accd = ev.tile([mc, 512], F16, tag=f"vad{vi}",
                                           name=f"accd{vi}")
                            t0 = dch[0]
                            bias0 = (dwb_s[:mc, mi:mi + 1] if with_bias
                                     else None)
                            in0, sc = ((tmps[t0][:], 1.0) if t0 in tmps
                                       else (win_of(t0), dcol_of(t0)))
                            o = accd[:] if t0 in tmps else \
                                accd[:].rearrange("p (h w) -> p h w", h=8)
                            if bias0 is None:
                                nc.vector.tensor_scalar(o, in0, sc, None,
                                                        AL.mult)
                            else:
                                nc.vector.tensor_scalar(o, in0, sc, bias0,
                                                        AL.mult, AL.add)
                            for t in dch[1:]:
                                nc.vector.tensor_tensor(
                                    accd[:], accd[:], tmps[t][:], AL.add)
                            parts.append(('d', accd))
                        # Pool chain: tt of the first two tmps, then adds;
                        # returned as ('p', tile) so vmerge may extend the
                        # pool chain with the psum eviction
                        if len(pch) == 1:
                            parts.append(('p', tmps[pch[0]]))
                        elif pch:
                            accp = ev.tile([mc, 512], F16, tag=f"vap{vi}",
                                           name=f"accp{vi}")
                            nc.gpsimd.tensor_tensor(
                                accp[:], tmps[pch[0]][:], tmps[pch[1]][:],
                                AL.add)
                            for t in pch[2:]:
                                nc.gpsimd.tensor_tensor(
                                    accp[:], accp[:], tmps[t][:], AL.add)
                            parts.append(('p', accp))
                        return parts

                    def vmerge(vi, mi, mc, parts, ps, n_pe_taps):
                        dst = v_t[vi][:, co:co + 512]
                        bias = dwb_s[:mc, mi:mi + 1]
                        if n_pe_taps and not parts:
                            nc.scalar.activation(dst, ps[:mc, :512],
                                                 AF.Identity, bias=bias)
                            return
                        if n_pe_taps:
                            tmpm = ev.tile([mc, 512], F16, tag=f"vmrg{vi}",
                                           name=f"mrg{vi}")
                            nc.scalar.activation(tmpm[:], ps[:mc, :512],
                                                 AF.Identity, bias=bias)
                            parts = parts + [('m', tmpm)]
                        tiles = [t for _, t in parts]
                        while len(tiles) > 2:
                            x = ev.tile([mc, 512], F16, tag=f"vmx{vi}",
                                        name=f"mx{vi}")
                            nc.vector.tensor_tensor(x[:], tiles[0][:],
                                                    tiles[1][:], AL.add)
                            tiles = [x] + tiles[2:]
                        if len(tiles) == 2:
                            nc.vector.tensor_tensor(dst, tiles[0][:],
                                                    tiles[1][:], AL.add)
                        elif n_pe_taps or parts[0][0] == 'd':
                            # bias already inside tmpm / the DVE chain init
                            nc.vector.tensor_scalar(dst, tiles[0][:], 1.0,
                                                    None, AL.mult)
                        else:
                            nc.vector.tensor_scalar(dst, tiles[0][:], 1.0,
                                                    bias, AL.mult, AL.add)

                    tvalid = lambda dt: dt in planes
                    last = (chunk == 2 * T - 1)
                    sp3 = M3_SPLIT_LAST if last else M3_SPLIT
                    sp4 = M4_SPLIT_LAST if last else M4_SPLIT
                    # m3: 27 diag taps
                    taps3 = [t for t in M3_TAPS if tvalid(t[0])]
                    pe3, seq3 = vsplit(taps3, sp3)
                    src3 = lambda dt: slabs[t_o + 1 + dt][3]
                    parts3 = vtap_nonpe(0, 3, 128, src3, seq3,
                                        with_bias=not pe3)
                    ps3 = None
                    if pe3:
                        ps3 = psp.tile([128, 512], F32, tag="mm")
                        for j, (dt, dh, dwv) in enumerate(pe3):
                            ti = TAPS.index((dt, dh, dwv))
                            nc.tensor.matmul(
                                ps3[:128, :512],
                                vd3[:, 128 * ti:128 * (ti + 1)],
                                win(src3(dt), 1 + dh, dwv, 128),
                                start=(j == 0), stop=(j == len(pe3) - 1))
                    vmerge(0, 3, 128, parts3, ps3, len(pe3))
                    # m4: PE pairs on the dup'd ring + split singles
                    pairs4 = [(dt, dw) for dt, dw in M4_PAIRS if tvalid(dt)]
                    sing4 = [t for t in M4_SINGLES if tvalid(t[0])]
                    pe4, seq4 = vsplit(sing4, sp4)
                    src4 = lambda dt: slabs[t_o + 1 + dt][4][0:64, :]
                    parts4 = vtap_nonpe(1, 4, 64, src4, seq4)
                    ps4 = psp.tile([128, 512], F32, tag="mm")
                    n4 = len(pairs4) + len(pe4)
                    pd4 = ring4.ap[0]
                    for j, (dt, dwv) in enumerate(pairs4):
                        pidx = M4_PAIRS.index((dt, dwv))
                        src = slabs[t_o + 1 + dt][4]
                        rhs = AP(tensor=src.tensor,
                                 offset=src.offset + 1 + dwv,
                                 ap=bass_rust.VecI64Pair(
                                     [[pd4[0], 128], [WP, 8], [1, 64]]))
                        nc.tensor.matmul(
                            ps4[:64, :512],
                            vd4p_s[:, 64 * pidx:64 * (pidx + 1)], rhs,
                            start=(j == 0), stop=(j == n4 - 1))
                    for j, (dt, dh, dwv) in enumerate(pe4):
                        si = M4_SINGLES.index((dt, dh, dwv))
                        nc.tensor.matmul(
                            ps4[:64, :512], vd4[:, 64 * si:64 * (si + 1)],
                            win(src4(dt), 1 + dh, dwv, 64),
                            start=(len(pairs4) + j == 0),
                            stop=(len(pairs4) + j == n4 - 1))
                    vmerge(1, 4, 64, parts4, ps4, n4)

            # per-batch norms + softmax + attn@v + proj (emitted after each
            # batch's AllReduce so batch 0's tail overlaps batch 1's dw).
            # Partition layout: (head, c) pairs in two 96-partition tiles,
            # so softmax sums fuse into the Exp via accum_out.
            def attn_batch(b):
                g8 = gram_out[8 * b:8 * (b + 1)]
                # kk diags for all 8 heads in one DMA; distributed to the
                # (h,c) partition layout by a tiny selector matmul
                kk8 = sp.tile([8, 24], F32, tag="kk8", name="kk8")
                nc.scalar.dma_start(
                    out=kk8[:],
                    in_=AP(tensor=g8.tensor,
                           offset=g8.offset + 24 * 48 + 24,
                           ap=bass_rust.VecI64Pair([[2304, 8], [49, 24]])))
                # 2^-12 prescale keeps the gram diag inside f16 range for
                # the selector matmul; compensated via the exp bias below
                kk8h = sp.tile([8, 24], F16, tag="kk8h", name="kk8h")
                nc.scalar.activation(kk8h[:], kk8[:], AF.Identity,
                                     scale=2.0 ** -12)
                a_rhs = sp.tile([24, 8, 24], F16, tag=f"ar{b}",
                                name="a_rhs")
                qq2 = sp.tile([96, 2], F32, tag="qq2", name="qq2")
                qkf2 = sp.tile([96, 2, 24], F32, tag="qkf2", name="qkf2")
                for hh in range(2):
                    goff = g8.offset + hh * 4 * 2304
                    nc.sync.dma_start(
                        out=qq2[:, hh:hh + 1],
                        in_=AP(tensor=g8.tensor, offset=goff,
                               ap=bass_rust.VecI64Pair(
                                   [[2304, 4], [49, 24]])))
                    nc.scalar.dma_start(
                        out=qkf2[:, hh, :],
                        in_=AP(tensor=g8.tensor, offset=goff + 24,
                               ap=bass_rust.VecI64Pair(
                                   [[2304, 4], [48, 24], [1, 24]])))
                for hh in range(2):
                    qq, qkf = qq2[:, hh:hh + 1], qkf2[:, hh, :]
                    # kk replicated to (h,c) partitions via selector matmul
                    pk = psav.tile([128, 512], F32, tag="av")
                    nc.tensor.matmul(pk[:96, :24],
                                     hsel_s[:, 96 * hh:96 * (hh + 1)],
                                     kk8h[:], start=True, stop=True)
                    # rq = temp * qq^-1/2 = exp(-ln(qq)/2 + ln temp);
                    # rk likewise (norms are sums of squares, > 0)
                    lq = sp.tile([96, 1], F32, tag=f"lq{hh}", name="lq")
                    nc.scalar.activation(lq[:], qq, AF.Ln)
                    rq = sp.tile([96, 1], F32, tag=f"rq{hh}", name="rq")
                    nc.scalar.activation(rq[:], lq[:], AF.Exp,
                                         bias=tln[b][hh][:, 0:1], scale=-0.5)
                    lk = sp.tile([96, 24], F32, tag=f"lk{hh}", name="lk")
                    nc.scalar.activation(lk[:], pk[:96, :24], AF.Ln)
                    rk = sp.tile([96, 24], F32, tag=f"rk{hh}", name="rk")
                    # kk = pk * 2^12  ->  kk^-1/2 = exp(-ln(pk)/2 - 6 ln 2)
                    nc.scalar.activation(rk[:], lk[:], AF.Exp, scale=-0.5,
                                         bias=kbias[:96, 0:1])
                    # logits, exp (+fused row-sum), renorm; |logit| <= temp
                    # so exp() is overflow-safe without max subtraction
                    a1 = sp.tile([96, 24], F32, tag=f"a1{hh}", name="a1")
                    nc.vector.tensor_scalar(a1[:], qkf, rq[:, 0:1], None,
                                            AL.mult)
                    nc.vector.tensor_tensor(a1[:], a1[:], rk[:], AL.mult)
                    ex = sp.tile([96, 24], F32, tag=f"ex{hh}", name="ex")
                    sm = sp.tile([96, 1], F32, tag=f"sm{hh}", name="sm")
                    nc.scalar.activation(ex[:], a1[:], AF.Exp,
                                         accum_out=sm[:])
                    rs = sp.tile([96, 1], F32, tag=f"rs{hh}", name="rs")
                    nc.vector.reciprocal(rs[:], sm[:])
                    a16 = sp.tile([96, 24], F16, tag=f"a16{hh}", name="a16")
                    nc.vector.tensor_scalar(a16[:], ex[:], rs[:, 0:1], None,
                                            AL.mult)
                    # restage this half's heads while the other half's
                    # softmax math runs (hides the HWDGE serialization)
                    for h4 in range(4):
                        q = nc.sync if h4 % 2 == 0 else nc.scalar
                        q.dma_start(
                            out=a_rhs[:, 4 * hh + h4, :],
                            in_=a16[24 * h4:24 * (h4 + 1), :])

                # W2 = proj @ blockdiag(attn): per head a [24ch, 24] matmul
                # with the host-restaged projr lhsT, then transpose to get
                # the k-major lhsT for the fused (attn@v+proj) stage
                w2m = []
                for mi, (mo, mc) in enumerate(KTILES):
                    pw = psav.tile([128, 512], F32, tag="av")
                    for h in range(HEADS):
                        nc.tensor.matmul(
                            pw[:mc, 24 * h:24 * (h + 1)],
                            projr_s[:, 192 * h + mo:192 * h + mo + mc],
                            a_rhs[:, h, :],
                            start=(h == 0), stop=(h == HEADS - 1))
                    wm = sp.tile([mc, DIM], F16, tag=f"w2m{b}_{mi}",
                                 name="wm")
                    nc.scalar.activation(wm[:], pw[:mc, :DIM], AF.Identity)
                    w2m.append(wm)
                w2T = []
                for ki, (ko, kc) in enumerate(KTILES):
                    wt = sp.tile([kc, DIM], F16, tag=f"w2T{b}_{ki}",
                                 name="wt")
                    for mi, (mo, mc) in enumerate(KTILES):
                        pt = psav.tile([128, 512], F32, tag="av")
                        ptv = pt.bitcast(F16)[:kc, :mc]
                        nc.tensor.transpose(
                            ptv, w2m[mi][:, ko:ko + kc], ident16[:mc, :mc])
                        nc.scalar.activation(wt[:, mo:mo + mc], ptv,
                                             AF.Identity)
                    w2T.append(wt)
                return w2T

            def av_proj(w2T, chunks, pool=None):
                # chunks processed in pairs sharing one output DMA per
                # mtile (HWDGE descriptor generation is the tail limiter)
                chunks = list(chunks)
                pairs = [chunks[i:i + 2] for i in range(0, len(chunks), 2)]
                for pi, pair in enumerate(pairs):
                    for mi, (mo, mc) in enumerate(KTILES):
                        of = op.tile([128, len(pair), 512], F32,
                                     tag=f"of{mi}")
                        for ci, chunk in enumerate(pair):
                            co = 512 * chunk
                            ps = ((pool or psav)
                                  .tile([128, 512], F32,
                                        tag="av" if pool is None else "mm"))
                            for ki in range(2):
                                nc.tensor.matmul(
                                    ps[:mc, :], w2T[ki][:, mo:mo + mc],
                                    v_t[ki][:, co:co + 512],
                                    start=(ki == 0), stop=(ki == 1))
                            if (ci + mi) % 2 == 0:
                                nc.vector.tensor_scalar(
                                    of[:mc, ci, :], ps[:mc, :],
                                    projb_s[:mc, mi:mi + 1], None, AL.add)
                            else:
                                nc.scalar.activation(
                                    of[:mc, ci, :], ps[:mc, :], AF.Identity,
                                    bias=projb_s[:mc, mi:mi + 1])
                        co0 = 512 * pair[0]
                        q = nc.sync if mi == 0 else nc.scalar
                        q.dma_start(
                            out=out[mo:mo + mc,
                                    co0:co0 + 512 * len(pair)],
                            in_=of[:mc, 0:len(pair), :])

            _gt = psg.tile([48, 384], F32, tag="gram", name="gram")
            gps = [_gt, _gt]
            # emission order == per-engine execution order (engines run
            # their streams in order), so nothing that waits on a collective
            # may be emitted ahead of ready work:
            #  b0 dw -> collective(0) -> b1 dw chunk 0 -> b0 softmax/W2 +
            #  av(0, 0..5) -> b1 dw rest -> collective(1) -> av(0, 5..8)
            #  (fills the collective-1 latency) -> b1 softmax/W2 -> av(1)
            w2T0 = None
            for b in range(B):
                slabs = {}
                for t_ in (1, 2, 3):
                    qkv_slab(b, t_, slabs)
                    if b == 0 and t_ == 1:
                        load_late_weights()
                for t_o in range(T):
                    if t_o + 4 <= T:
                        qkv_slab(b, t_o + 4, slabs)
                    dw_chunk(b, t_o, slabs)
                    # batch-0 softmax + spread-out av chunks ride along
                    # batch-1's dw stream (one av chunk per dw chunk)
                    if b == 1:
                        if t_o == 1:
                            w2T0 = attn_batch(0)
                        if t_o in (2, 4):
                            av_proj(w2T0, [t_o - 2, t_o - 1])
                gram_chunk(b, b * T + T - 1)
                gs = ev.tile([48, 384], F32, tag="gs")
                nc.vector.tensor_copy(gs[:], gps[b][:])
                nc.sync.dma_start(
                    out=gram_in[8 * b:8 * (b + 1)].rearrange(
                        "g c d -> c g d"),
                    in_=gs[:].rearrange("c (g d) -> c g d", g=8))
                nc.gpsimd.collective_compute(
                    "AllReduce", AL.add,
                    replica_groups=[list(range(NCORES))],
                    ins=[gram_in[8 * b:8 * (b + 1)]],
                    outs=[gram_out[8 * b:8 * (b + 1)]])
            av_proj(w2T0, range(4, T))
            w2T1 = attn_batch(1)
            # the dw stream is drained here; reuse its psum banks so the
            # tail av pipeline isn't throttled by psav's 2 banks
            av_proj(w2T1, range(T, 2 * T), pool=psp)

    nc.compile()
    return nc


def _prep_inputs(x, qkv_w, qkv_b, dw_w, dw_b, temperature, proj_w, proj_b):
    """Host-side prep: per-core padded fp16 slabs + shared weights."""
    x = np.asarray(x, np.float32)
    b_, c_, t_, h_, w_ = x.shape  # 2, 192, 8, 64, 64
    qkv_w2 = np.asarray(qkv_w, np.float32).reshape(C3, DIM)
    dw_w2 = np.asarray(dw_w, np.float32).reshape(C3, 27)
    proj_w2 = np.asarray(proj_w, np.float32).reshape(DIM, DIM)
    # permute qkv channels: [q_h0, k_h0, q_h1, k_h1, ..., v] so each head's
    # (q,k) columns are adjacent after transpose (contiguous gram operands)
    perm = []
    for h in range(HEADS):
        perm.extend(range(HD * h, HD * (h + 1)))          # q_h
        perm.extend(range(DIM + HD * h, DIM + HD * (h + 1)))  # k_h
    perm.extend(range(2 * DIM, 3 * DIM))                  # v unchanged
    perm = np.array(perm)
    qkv_w2 = qkv_w2[perm]
    dw_w2 = dw_w2[perm]
    qkv_b = np.asarray(qkv_b, np.float32)[perm]
    dw_b = np.asarray(dw_b, np.float32)[perm]

    import ml_dtypes
    FP8 = ml_dtypes.float8_e4m3

    qkvb_h = np.zeros((128, 5), np.float32)
    dwb_h = np.zeros((128, 5), np.float32)
    for mi, (mo, mc) in enumerate(MTILES):
        s = ASCALE if mi < 3 else 1.0  # qk slab evicted as fp8(ASCALE*psum)
        qkvb_h[:mc, mi] = np.asarray(qkv_b, np.float32)[mo:mo + mc] * s
        dwb_h[:mc, mi] = np.asarray(dw_b, np.float32)[mo:mo + mc]
    qkvb_h[64:128, 4] = qkvb_h[0:64, 4]  # dup half of the m4 ring

    # fp8 DoubleRow diag-pair tiles for q/k (values WSCALE*d, fp8-rounded)
    tap_i = {tap: i for i, tap in enumerate(TAPS)}
    qkd = np.zeros((128, 3 * NQK_TILES * 256), FP8)
    d8 = (WSCALE * dw_w2).astype(FP8)  # [576, 27]
    rng = np.arange(128)

    def put(mi, j, slot, tap):
        base = 256 * (mi * NQK_TILES + j) + 128 * slot
        qkd[rng, base + rng] = d8[128 * mi + rng, tap_i[tap]]

    for mi in range(3):
        for dt in (-1, 0, 1):
            for dw in (-1, 0, 1):
                put(mi, (dt + 1) * 3 + dw + 1, 0, (dt, -1, dw))
                put(mi, (dt + 1) * 3 + dw + 1, 1, (dt, 1, dw))
        for dw in (-1, 0, 1):
            put(mi, 9 + dw + 1, 0, (0, 0, dw))
            put(mi, 12 + dw + 1, 0, (-1, 0, dw))
            put(mi, 12 + dw + 1, 1, (1, 0, dw))
            put(mi, 15 + dw + 1, 0, (0, 0, dw))
            put(mi, 15 + dw + 1, 1, (1, 0, dw))
            put(mi, 18 + dw + 1, 0, (-1, 0, dw))
            put(mi, 18 + dw + 1, 1, (0, 0, dw))

    # exact fp16 diag tiles for v
    vd3_h = np.zeros((128, 27 * 128), np.float16)
    vd4_h = np.zeros((64, 9 * 64), np.float16)
    r64 = np.arange(64)
    for ti in range(27):
        vd3_h[rng, 128 * ti + rng] = dw_w2[384 + rng, ti].astype(np.float16)
    for si, tap in enumerate(M4_SINGLES):
        vd4_h[r64, 64 * si + r64] = \
            dw_w2[512 + r64, tap_i[tap]].astype(np.float16)
    # m4 dup-ring pair tiles: rows 0..63 diag of tap (dt,-1,dw), rows
    # 64..127 diag of tap (dt,+1,dw)
    vd4p_h = np.zeros((128, 9 * 64), np.float16)
    r64 = np.arange(64)
    for pidx, (dt, dwv) in enumerate(
            [(dt, dwv) for dt in (0, -1, 1) for dwv in (-1, 0, 1)]):
        a = tap_i[(dt, -1, dwv)]
        bb = tap_i[(dt, 1, dwv)]
        vd4p_h[r64, 64 * pidx + r64] = dw_w2[512 + r64, a].astype(np.float16)
        vd4p_h[64 + r64, 64 * pidx + r64] = \
            dw_w2[512 + r64, bb].astype(np.float16)
    # fp16 v-conv lhsT per ktile: cols 0..127 mtile3, 128..255 mtile4 dup'd
    wv16_h = np.zeros((128, 512), np.float16)
    for ki, (ko, kc) in enumerate(KTILES):
        wv16_h[:kc, 256 * ki:256 * ki + 128] = \
            qkv_w2[384:512, ko:ko + kc].T.astype(np.float16)
        wv16_h[:kc, 256 * ki + 128:256 * ki + 192] = \
            qkv_w2[512:576, ko:ko + kc].T.astype(np.float16)
        wv16_h[:kc, 256 * ki + 192:256 * ki + 256] = \
            qkv_w2[512:576, ko:ko + kc].T.astype(np.float16)

    # per-channel f32 v-diag columns for the Pool-engine taps
    vdws_h = np.zeros((128, 54), np.float32)
    vdws_h[:, 0:27] = dw_w2[384:512]
    vdws_h[:64, 27:54] = dw_w2[512:576]

    # fp8 q/k 1x1-conv weights [96, 2, 128] per qk mtile (lhsT layout:
    # W[p, j, m] = qkv_w[out=mo+m, in=p+96j])
    wq8_h = np.zeros((96, 3 * 256), FP8)
    for mi in range(3):
        for j in range(2):
            blk = qkv_w2[128 * mi:128 * (mi + 1), 96 * j:96 * (j + 1)].T
            wq8_h[:, 256 * mi + 128 * j:256 * mi + 128 * (j + 1)] = \
                blk.astype(FP8)
    projr_h = np.zeros((HD, HEADS * DIM), np.float16)
    for h in range(HEADS):
        projr_h[:, DIM * h:DIM * (h + 1)] = \
            proj_w2[:, HD * h:HD * (h + 1)].T.astype(np.float16)
    projb_h = np.zeros((128, 2), np.float32)
    projb_h[:128, 0] = np.asarray(proj_b, np.float32)[0:128]
    projb_h[:64, 1] = np.asarray(proj_b, np.float32)[128:192]
    tv = np.asarray(temperature, np.float32).reshape(HEADS, 1)
    temp_h = np.concatenate(
        [np.repeat(tv, 2, axis=1),
         np.repeat(np.log(np.maximum(tv, 1e-30)), 2, axis=1)],
        axis=1).astype(np.float32)  # [head, (temp*2, ln temp*2)]
    hsel_h = np.zeros((8, 192), np.float16)
    for h in range(8):
        hh, h4 = h // 4, h % 4
        hsel_h[h, 96 * hh + 24 * h4:96 * hh + 24 * (h4 + 1)] = 1.0

    in_maps = []
    for i in range(NCORES):
        # padded slab [b, t10, h10, w66], h rows 8i-1 .. 8i+9 clamped->zero
        xs = np.zeros((b_, TP, HP, XW, c_), np.float32)
        hlo, hhi = 8 * i - 1, 8 * i + 9
        slo, shi = max(0, hlo), min(h_, hhi)
        # x [b,c,t,h,w] -> [b,t,h,w,c]
        xt = x[:, :, :, slo:shi, :].transpose(0, 2, 3, 4, 1)
        xs[:, 1:9, (slo - hlo):(slo - hlo) + (shi - slo), 1:65, :] = xt
        xflat = xs.reshape(b_ * TP * HP * XW, c_)
        x16 = np.ascontiguousarray(xflat.T).astype(np.float16)
        x8_h = np.ascontiguousarray(
            xflat.T.reshape(2, 96, NPADTOK).transpose(1, 0, 2)
            .reshape(96, 2 * NPADTOK)).astype(FP8)
        in_maps.append({
            "x16": x16, "x8": x8_h, "wq8d": wq8_h, "vdws": vdws_h,
            "wv16d": wv16_h, "vdiag4p": vd4p_h,
            "qkvb": qkvb_h, "qkdiag": qkd,
            "vdiag3": vd3_h, "vdiag4": vd4_h,
            "dwb": dwb_h, "projr": projr_h, "projb": projb_h,
            "temp": temp_h, "hsel": hsel_h,
        })
    return in_maps


def _get_runner():
    """Build once; return a persistent sharded-jit callable (the per-call
    closure in bass2jax.run_bass_via_pjrt defeats jax's jit cache)."""
    if "runner" in _CACHE:
        return _CACHE["runner"]
    import jax
    for flag, val in [("jax_compilation_cache_dir", "/tmp/jax_kernel_cache"),
                      ("jax_persistent_cache_min_compile_time_secs", 1.0),
                      ("jax_persistent_cache_min_entry_size_bytes", 0)]:
        try:
            jax.config.update(flag, val)
        except Exception:
            pass
    import jax.numpy as jnp
    from jax.sharding import Mesh, PartitionSpec
    from jax.experimental.shard_map import shard_map
    import concourse.mybir as mybir
    from concourse import bass2jax

    nc = _build()
    bass2jax.install_neuronx_cc_hook()

    partition_name = (nc.partition_id_tensor.name
                      if nc.partition_id_tensor else None)
    in_names, out_names, out_avals, zero_shapes = [], [], [], []
    for alloc in nc.m.functions[0].allocations:
        if not isinstance(alloc, mybir.MemoryLocationSet):
            continue
        name = alloc.memorylocations[0].name
        if alloc.kind == "ExternalInput":
            if name != partition_name:
                in_names.append(name)
        elif alloc.kind == "ExternalOutput":
            shape = tuple(alloc.tensor_shape)
            dtype = mybir.dt.np(alloc.dtype)
            out_names.append(name)
            out_avals.append(jax.core.ShapedArray(shape, dtype))
            zero_shapes.append((shape, dtype))
    n_params = len(in_names)
    all_names = in_names + out_names
    if partition_name is not None:
        all_names.append(partition_name)

    def _body(*args):
        operands = list(args)
        if partition_name is not None:
            operands.append(bass2jax.partition_id_tensor())
        outs = bass2jax._bass_exec_p.bind(
            *operands, out_avals=tuple(out_avals), in_names=tuple(all_names),
            out_names=tuple(out_names), lowering_input_output_aliases=(),
            sim_require_finite=True, sim_require_nnan=True, nc=nc)
        return tuple(outs)

    devices = jax.devices()[:NCORES]
    mesh = Mesh(np.asarray(devices), ("core",))
    n_outs = len(out_names)
    sharded = jax.jit(
        shard_map(_body, mesh=mesh,
                  in_specs=(PartitionSpec("core"),) * (n_params + n_outs),
                  out_specs=(PartitionSpec("core"),) * n_outs,
                  check_rep=False),
        donate_argnums=tuple(range(n_params, n_params + n_outs)),
        keep_unused=True)

    def run(in_maps):
        concat_in = [np.concatenate([in_maps[c][nm] for c in range(NCORES)],
                                    axis=0) for nm in in_names]
        concat_zeros = [np.zeros((NCORES * s[0], *s[1:]), dt)
                        for s, dt in zero_shapes]
        out_arrs = sharded(*concat_in, *concat_zeros)
        return [
            {nm: np.asarray(out_arrs[i]).reshape(NCORES, *out_avals[i].shape)[c]
             for i, nm in enumerate(out_names)}
            for c in range(NCORES)]

    _CACHE["runner"] = run
    return run


def kernel(x, qkv_w, qkv_b, dw_w, dw_b, temperature, proj_w, proj_b):
    run = _get_runner()
    in_maps = _prep_inputs(x, qkv_w, qkv_b, dw_w, dw_b, temperature,
                           proj_w, proj_b)
    results = run(in_maps)
    b_, c_, t_, h_, w_ = np.asarray(x).shape
    outf = np.empty((b_, c_, t_, h_, w_), np.float32)
    for i in range(NCORES):
        o = results[i]["out"].reshape(c_, b_, t_, H, w_)
        outf[:, :, :, 8 * i:8 * i + 8, :] = o.transpose(1, 0, 2, 3, 4)
    return outf

